# revision 29
# baseline (speedup 1.0000x reference)
"""Trainium2 Bass kernel for nn_MiniMHCLM (moe_routing).

Strategy (8 NeuronCores, SPMD, no collectives):
  - vocab-sharded head matmul: core i holds w_head rows [i*VS:(i+1)*VS]
    (host-sliced, zero-padded to uniform VS) transposed to k-major bf16;
    it computes logits for all 4096 tokens x its vocab slice and the host
    concatenates along vocab.
  - token embeddings are pre-gathered AND pre-transposed on the host into
    xT [K, NT] bf16 (numerically identical to embed[ids].astype(bf16)),
    so the device pipeline runs fully k-major with zero PE transposes of x
    and no indirect DMA.
  - per-token coeffs: phi-stationary matmul gives logits^T [24, T] plus a
    ones-matmul row of sum(x^2); one small PE transpose per 128-token
    chunk moves both to token-major for the RMS scale, sigmoid/exp and
    the Sinkhorn iterations (DVE/ACT, batched per token group).
  - mixing runs transposed: per-token coeffs are PE-transposed back to
    [24, T], bounced through a DRAM scratch row and broadcast across
    partitions with a stride-0 DMA read; x_merge^T is then built with
    DVE/GPSIMD multiply-adds and fed straight into the head matmul as
    the stationary operand.
  - head matmul in bf16 with fp32 PSUM; PSUM evacuated by ACT copies to
    bf16 and DMA'd to DRAM bf16 (host converts to fp32).
  - software pipeline over token groups with a warm-up ramp
    (128/256/512... tokens): group g's head matmuls overlap group g+1's
    coeff pipeline and group g+2's logits, keeping the PE stream dense.
"""

import numpy as np

HC, C, TMAX = 4, 256, 8
RMS_EPS, PRE_EPS, SINK_EPS, POST_MULT = 1e-6, 1e-4, 1e-6, 2.0
VOCAB = 50257
B, S = 2, 2048
K = HC * C            # 1024
M = HC * HC + 2 * HC  # 24
NKC = K // 128        # 8 k-chunks
NCORES = 8
NT = B * S            # 4096
VS = 6283             # vocab rows per core (8*6283 = 50264 >= 50257)
VW = 512
NV = (VS + VW - 1) // VW          # 13 head tiles (12x512 + 139)
SCS = [128, 256] + [512] * 7 + [128]   # token-group ramp, sum = NT
OFF = [sum(SCS[:i]) for i in range(len(SCS))]
NG = len(SCS)
assert sum(SCS) == NT


def _build():
    from contextlib import ExitStack
    from concourse import bass, bacc, mybir
    import concourse.tile as tile
    from concourse.masks import make_identity

    f32 = mybir.dt.float32
    bf16 = mybir.dt.bfloat16
    AX = mybir.AxisListType
    OP = mybir.AluOpType
    AF = mybir.ActivationFunctionType

    nc = bacc.Bacc(target_bir_lowering=False)
    xt_p = nc.declare_dram_parameter("xt", [K, NT], bf16, False)
    wvt_p = nc.declare_dram_parameter("wvt", [K, VS], bf16, False)
    wit_p = nc.declare_dram_parameter("wit", [C, C], bf16, False)
    phi_p = nc.declare_dram_parameter("phi", [K, M], bf16, False)
    b_p = nc.declare_dram_parameter("b", [1, M], f32, False)
    al_p = nc.declare_dram_parameter("al", [1, 3], f32, False)
    out_p = nc.declare_dram_parameter("out", [NT, VS], bf16, True)

    with ExitStack() as ctx:
        tc = ctx.enter_context(tile.TileContext(nc))
        const = ctx.enter_context(tc.tile_pool(name="const", bufs=1))
        wtp = ctx.enter_context(tc.tile_pool(name="wtp", bufs=1))
        xtp = ctx.enter_context(tc.tile_pool(name="xtp", bufs=3))
        lgp = ctx.enter_context(tc.tile_pool(name="lgp", bufs=2))
        cfp = ctx.enter_context(tc.tile_pool(name="cfp", bufs=2))
        plp = ctx.enter_context(tc.tile_pool(name="plp", bufs=1))
        mxp = ctx.enter_context(tc.tile_pool(name="mxp", bufs=2))
        wkp = ctx.enter_context(tc.tile_pool(name="wkp", bufs=4))
        x2p = ctx.enter_context(tc.tile_pool(name="x2p", bufs=1))
        stp = ctx.enter_context(tc.tile_pool(name="stp", bufs=5))
        psh = ctx.enter_context(tc.tile_pool(name="psh", bufs=3, space="PSUM"))
        psa = ctx.enter_context(tc.tile_pool(name="psa", bufs=1, space="PSUM"))
        psb = ctx.enter_context(tc.tile_pool(name="psb", bufs=1, space="PSUM"))
        pst = ctx.enter_context(tc.tile_pool(name="pst", bufs=2, space="PSUM"))
        psf = ctx.enter_context(tc.tile_pool(name="psf", bufs=1, space="PSUM"))
        drp = ctx.enter_context(tc.tile_pool(name="drp", bufs=2, space="DRAM"))

        # ---------------- constants ----------------
        ident = const.tile([128, 128], bf16)
        make_identity(nc, ident[:])
        identf = const.tile([128, 128], f32)
        make_identity(nc, identf[:])

        cst = const.tile([128, 2], f32)
        nc.vector.memset(cst[:, 0:1], 0.0)
        nc.vector.memset(cst[:, 1:2], RMS_EPS)
        zero_b = cst[:, 0:1]
        eps_b = cst[:, 1:2]

        ones = const.tile([128, 1], bf16)
        nc.vector.memset(ones[:], 1.0)

        phi_sb = const.tile([128, NKC * M], bf16)
        for kc in range(NKC):
            nc.sync.dma_start(out=phi_sb[:, kc * M:(kc + 1) * M],
                              in_=phi_p[kc * 128:(kc + 1) * 128, :])
        b_bc = const.tile([128, M], f32)
        nc.sync.dma_start(out=b_bc[:], in_=b_p[0:1, :].to_broadcast([128, M]))
        al_bc = const.tile([128, 3], f32)
        nc.sync.dma_start(out=al_bc[:], in_=al_p[0:1, :].to_broadcast([128, 3]))

        # w_inner^T (k-major [c, o]) as 2 row bands
        wit_sb = const.tile([128, 2 * C], bf16)
        for h in range(2):
            nc.sync.dma_start(out=wit_sb[:, h * C:(h + 1) * C],
                              in_=wit_p[h * 128:(h + 1) * 128, :])

        # w_head^T slice, 8 k row bands
        wt_all = wtp.tile([128, NKC * VS], bf16, tag="wt_all")
        for kc in range(NKC):
            nc.sync.dma_start(out=wt_all[:, kc * VS:(kc + 1) * VS],
                              in_=wvt_p[kc * 128:(kc + 1) * 128, :])

        st = {}  # per-group live tiles

        # ---------------- pipeline stages ----------------
        def stage_lg(g):
            """xT DMA, phi logits^T + sumsq row, transpose to token-major,
            RMS scale + coeff activations + Sinkhorn -> coefs."""
            gt, t0 = SCS[g], OFF[g]
            nch = gt // 128
            xtg = xtp.tile([128, NKC * gt], bf16, tag="xtg", name=f"xtg{g}")
            for kc in range(NKC):
                nc.sync.dma_start(
                    out=xtg[:, kc * gt:(kc + 1) * gt],
                    in_=xt_p[kc * 128:(kc + 1) * 128, t0:t0 + gt])

            # squares for the RMS sum (bf16 is plenty for the mean)
            x2s = []
            for half in range(2):
                x2 = x2p.tile([128, 4 * gt], bf16, tag=f"x2{half}",
                              name=f"x2_{g}_{half}")
                sl = slice(half * 4 * gt, (half + 1) * 4 * gt)
                nc.gpsimd.tensor_tensor(
                    out=x2[:], in0=xtg[:, sl], in1=xtg[:, sl], op=OP.mult)
                x2s.append(x2)

            pslg = psa.tile([32, gt], f32, tag="pslg")
            psss = psb.tile([32, gt], f32, tag="psss")
            for kc in range(NKC):
                nc.tensor.matmul(
                    out=pslg[0:M, :],
                    lhsT=phi_sb[:, kc * M:(kc + 1) * M],
                    rhs=xtg[:, kc * gt:(kc + 1) * gt],
                    start=(kc == 0), stop=(kc == NKC - 1))
            for kc in range(NKC):
                nc.tensor.matmul(
                    out=psss[0:1, :],
                    lhsT=ones[:],
                    rhs=x2s[kc // 4][:, (kc % 4) * gt:(kc % 4 + 1) * gt],
                    start=(kc == 0), stop=(kc == NKC - 1))

            lgsb = lgp.tile([32, gt], f32, tag="lgsb", name=f"lgsb{g}")
            nc.scalar.copy(lgsb[0:M, :], pslg[0:M, :])
            ssq = lgp.tile([1, gt], f32, tag="ssq", name=f"ssq{g}")
            nc.scalar.copy(ssq[0:1, :], psss[0:1, :])

            # token-major [128, nch, 24] + per-token sumsq column
            lgtm = lgp.tile([128, nch * 32], f32, tag="lgtm", name=f"lgtm{g}")
            msq = lgp.tile([128, nch], f32, tag="msq", name=f"msq{g}")
            for tcx in range(nch):
                pT = pst.tile([128, 128], f32, tag="psT")
                nc.tensor.transpose(
                    out=pT[:, 0:M],
                    in_=lgsb[0:M, tcx * 128:(tcx + 1) * 128],
                    identity=identf[0:M, 0:M])
                nc.tensor.transpose(
                    out=pT[:, M:M + 1],
                    in_=ssq[0:1, tcx * 128:(tcx + 1) * 128],
                    identity=identf[0:1, 0:1])
                nc.scalar.copy(lgtm[:, tcx * 32:tcx * 32 + M], pT[:, 0:M])
                nc.scalar.copy(msq[:, tcx:tcx + 1], pT[:, M:M + 1])
            lgv = lgtm[:].rearrange("p (c w) -> p c w", w=32)

            # scl = 1/sqrt(mean+eps)
            scl = lgp.tile([128, nch], f32, tag="scl", name=f"scl{g}")
            nc.scalar.activation(out=scl[:], in_=msq[:],
                                 func=AF.Sqrt, scale=1.0 / K, bias=eps_b)
            nc.vector.reciprocal(scl[:], scl[:])
            for tcx in range(nch):
                nc.vector.tensor_scalar_mul(
                    lgv[:, tcx, 0:M], lgv[:, tcx, 0:M], scl[:, tcx:tcx + 1])
            nc.vector.tensor_tensor(
                out=lgv[:, :, 0:M], in0=lgv[:, :, 0:M],
                in1=b_bc[:][:, None, :].to_broadcast([128, nch, M]), op=OP.add)

            # coefs [128, nch, 24]: [0:16]=exp(res), [16:20]=h_pre,
            # [20:24]=h_post2
            coefs = cfp.tile([128, nch * M], f32, tag="coefs",
                             name=f"coefs{g}")
            cfv = coefs[:].rearrange("p (c m) -> p c m", m=M)
            nc.scalar.activation(out=cfv[:, :, 16:20], in_=lgv[:, :, 0:4],
                                 func=AF.Sigmoid, bias=zero_b,
                                 scale=al_bc[:, 0:1])
            nc.vector.tensor_scalar_add(cfv[:, :, 16:20], cfv[:, :, 16:20],
                                        PRE_EPS)
            nc.scalar.activation(out=cfv[:, :, 20:24], in_=lgv[:, :, 4:8],
                                 func=AF.Sigmoid, bias=zero_b,
                                 scale=al_bc[:, 1:2])
            nc.vector.tensor_scalar_mul(cfv[:, :, 20:24], cfv[:, :, 20:24],
                                        POST_MULT)
            nc.scalar.activation(out=cfv[:, :, 0:16], in_=lgv[:, :, 8:24],
                                 func=AF.Exp, bias=zero_b, scale=al_bc[:, 2:3])

            # batched Sinkhorn on cfv[:, :, 0:16].
            # SINK_EPS (1e-6 vs O(1) row sums) is dropped: it shifts the
            # result by ~1e-6 relative, far below the bf16 noise floor.
            mv4 = cfv[:, :, 0:16].rearrange("p c (o i) -> p c o i", i=4)
            mv4t = cfv[:, :, 0:16].rearrange("p c (o i) -> p c i o", i=4)
            for _ in range(TMAX):
                rs = wkp.tile([128, 4 * 4], f32, tag="rs")
                rsv = rs[:, 0:nch * 4].rearrange("p (c o) -> p c o", c=nch)
                nc.vector.tensor_reduce(rsv, mv4, axis=AX.X, op=OP.add)
                nc.vector.reciprocal(rs[:, 0:nch * 4], rs[:, 0:nch * 4])
                nc.vector.tensor_tensor(
                    out=mv4, in0=mv4,
                    in1=rsv[:, :, :, None].to_broadcast([128, nch, 4, 4]),
                    op=OP.mult)
                cs = wkp.tile([128, 4 * 4], f32, tag="cs")
                csv = cs[:, 0:nch * 4].rearrange("p (c i) -> p c i", c=nch)
                nc.vector.tensor_reduce(csv, mv4t, axis=AX.X, op=OP.add)
                nc.vector.reciprocal(cs[:, 0:nch * 4], cs[:, 0:nch * 4])
                nc.vector.tensor_tensor(
                    out=mv4, in0=mv4,
                    in1=csv[:, :, None, :].to_broadcast([128, nch, 4, 4]),
                    op=OP.mult)
            st[g] = dict(xtg=xtg, coefs=coefs)

        def stage_planes(g):
            """Transpose coefs back to [24, T]; bounce through DRAM and
            broadcast-read -> planes [128, 24*gt]."""
            gt = SCS[g]
            nch = gt // 128
            coefs = st[g]["coefs"]
            ctstg = cfp.tile([32, gt], bf16, tag="ctstg", name=f"ctstg{g}")
            for tcx in range(nch):
                pT = pst.tile([128, 128], f32, tag="psT")
                nc.tensor.transpose(
                    out=pT[0:M, 0:128],
                    in_=coefs[:, tcx * M:(tcx + 1) * M],
                    identity=identf[:, 0:128])
                nc.scalar.copy(
                    ctstg[0:M, tcx * 128:(tcx + 1) * 128], pT[0:M, 0:128])
            dtile = drp.tile([1, M * gt], bf16, tag="cfdram",
                             name=f"cfdram{g}")
            nc.sync.dma_start(
                out=dtile[0:1, :].rearrange("x (c t) -> (x c) t", c=M),
                in_=ctstg[0:M, :])
            planes = plp.tile([128, M * gt], bf16, tag="planes",
                              name=f"planes{g}")
            nc.sync.dma_start(
                out=planes[:],
                in_=dtile[0:1, :].to_broadcast([128, M * gt]))
            st[g]["planes"] = planes
            # x_in^T = sum_i h_pre[i] * x^T[i]  (2 half-chunks of c)
            xtg = st[g]["xtg"]
            xin = mxp.tile([128, 2 * gt], bf16, tag="xin", name=f"xin{g}")
            for h in range(2):
                seg = xin[:, h * gt:(h + 1) * gt]
                nc.vector.tensor_tensor(
                    out=seg, in0=xtg[:, h * gt:(h + 1) * gt],
                    in1=planes[:, 16 * gt:17 * gt], op=OP.mult)
                for i in range(1, HC):
                    tmp = wkp.tile([128, 512], bf16, tag="tmp")
                    nc.vector.tensor_tensor(
                        out=tmp[:, 0:gt],
                        in0=xtg[:, (i * 2 + h) * gt:(i * 2 + h + 1) * gt],
                        in1=planes[:, (16 + i) * gt:(17 + i) * gt],
                        op=OP.mult)
                    eng = nc.vector if i % 2 else nc.gpsimd
                    eng.tensor_add(seg, seg, tmp[:, 0:gt])
            st[g]["xin"] = xin

        def stage_fo(g):
            """f_out^T = w_inner @ x_in^T : 2 o-blocks x 2 c-halves."""
            gt = SCS[g]
            xin = st[g]["xin"]
            fo = mxp.tile([128, 2 * gt], bf16, tag="fo", name=f"fo{g}")
            for ob in range(2):
                pf = psf.tile([128, gt], f32, tag="psf")
                for h in range(2):
                    nc.tensor.matmul(
                        out=pf[:],
                        lhsT=wit_sb[:, h * C + ob * 128:h * C + (ob + 1) * 128],
                        rhs=xin[:, h * gt:(h + 1) * gt],
                        start=(h == 0), stop=(h == 1))
                nc.scalar.copy(fo[:, ob * gt:(ob + 1) * gt], pf[:])
            st[g]["fo"] = fo

        def stage_mix(g):
            """x_merge^T[kc] = sum_i res[o,i]*x^T[i,h] + post2[o]*f_out^T[h]"""
            gt = SCS[g]
            xtg, planes, fo = st[g]["xtg"], st[g]["planes"], st[g]["fo"]
            xmg = mxp.tile([128, NKC * gt], bf16, tag="xmg", name=f"xmg{g}")
            for kc in range(NKC):
                o, h = kc // 2, kc % 2
                seg = xmg[:, kc * gt:(kc + 1) * gt]
                nc.vector.tensor_tensor(
                    out=seg, in0=xtg[:, h * gt:(h + 1) * gt],
                    in1=planes[:, (o * 4) * gt:(o * 4 + 1) * gt], op=OP.mult)
                for i in range(1, HC):
                    tmp = wkp.tile([128, 512], bf16, tag="tmp")
                    nc.vector.tensor_tensor(
                        out=tmp[:, 0:gt],
                        in0=xtg[:, (i * 2 + h) * gt:(i * 2 + h + 1) * gt],
                        in1=planes[:, (o * 4 + i) * gt:(o * 4 + i + 1) * gt],
                        op=OP.mult)
                    eng = nc.vector if i % 2 else nc.gpsimd
                    eng.tensor_add(seg, seg, tmp[:, 0:gt])
                tmp = wkp.tile([128, 512], bf16, tag="tmp")
                nc.vector.tensor_tensor(
                    out=tmp[:, 0:gt], in0=fo[:, h * gt:(h + 1) * gt],
                    in1=planes[:, (20 + o) * gt:(21 + o) * gt], op=OP.mult)
                nc.gpsimd.tensor_add(seg, seg, tmp[:, 0:gt])
            st[g]["xmg"] = xmg

        def head_chunk(g, tcx):
            gt = SCS[g]
            xmg = st[g]["xmg"]
            t0 = OFF[g] + tcx * 128
            stg = None
            for v in range(NV):
                w = min(VW, VS - v * VW)
                ph = psh.tile([128, VW], f32, tag="psh")
                for kc in range(NKC):
                    nc.tensor.matmul(
                        out=ph[:, 0:w],
                        lhsT=xmg[:, kc * gt + tcx * 128:
                                 kc * gt + (tcx + 1) * 128],
                        rhs=wt_all[:, kc * VS + v * VW:kc * VS + v * VW + w],
                        start=(kc == 0), stop=(kc == NKC - 1))
                # pair two v-tiles per staging tile / output DMA
                half = v % 2
                if half == 0:
                    stg = stp.tile([128, 2 * VW], bf16, tag="stg")
                nc.scalar.copy(stg[:, half * VW:half * VW + w], ph[:, 0:w])
                if half == 1 or v == NV - 1:
                    v0 = v - half
                    ww = min(2 * VW, VS - v0 * VW)
                    nc.sync.dma_start(
                        out=out_p[t0:t0 + 128, v0 * VW:v0 * VW + ww],
                        in_=stg[:, 0:ww])

        # ---------------- emission (software pipeline) ----------------
        stage_lg(0)
        stage_lg(1)
        stage_planes(0)
        stage_fo(0)
        stage_mix(0)
        for g in range(NG):
            nch = SCS[g] // 128
            inserts = {0: [], 1: [], 2: []}
            if g + 1 < NG:
                inserts[0].append(("planes", g + 1))
                inserts[min(1, nch - 1)].append(("fo", g + 1))
                inserts[min(1, nch - 1)].append(("mix", g + 1))
            if g + 2 < NG:
                inserts[min(2, nch - 1)].append(("lg", g + 2))
            for tcx in range(nch):
                head_chunk(g, tcx)
                for kind, gg in inserts.get(tcx, []):
                    if kind == "planes":
                        stage_planes(gg)
                    elif kind == "fo":
                        stage_fo(gg)
                    elif kind == "mix":
                        stage_mix(gg)
                    else:
                        stage_lg(gg)
            del st[g]

    if not nc.is_finalized():
        nc.finalize()
    return nc


_NC_CACHE = {}


def _get_nc():
    if "nc" not in _NC_CACHE:
        _NC_CACHE["nc"] = _build()
    return _NC_CACHE["nc"]


def _make_in_maps(input_ids, embed, w_inner, w_head, phi, b,
                  alpha_pre, alpha_post, alpha_res):
    import ml_dtypes
    bf = ml_dtypes.bfloat16

    ids = np.asarray(input_ids).reshape(-1).astype(np.int64)
    x = np.asarray(embed)[ids].astype(bf)                 # [NT, K]
    xt = np.ascontiguousarray(x.T)                        # [K, NT]
    phi_np = np.ascontiguousarray(np.asarray(phi).astype(bf))
    wit = np.ascontiguousarray(np.asarray(w_inner).astype(bf).T)  # [c, o]
    b_np = np.ascontiguousarray(np.asarray(b, dtype=np.float32).reshape(1, M))
    al = np.array([[np.asarray(alpha_pre).reshape(-1)[0],
                    np.asarray(alpha_post).reshape(-1)[0],
                    np.asarray(alpha_res).reshape(-1)[0]]], dtype=np.float32)
    wh = np.asarray(w_head).astype(bf)                    # [VOCAB, K]

    in_maps = []
    for i in range(NCORES):
        sl = wh[i * VS:(i + 1) * VS]                      # [<=VS, K]
        wvt = np.zeros((K, VS), bf)
        wvt[:, :sl.shape[0]] = sl.T
        in_maps.append(dict(xt=xt, wvt=np.ascontiguousarray(wvt),
                            wit=wit, phi=phi_np, b=b_np, al=al))
    return in_maps


def _run(in_maps, trace=False):
    from concourse.bass_utils import run_bass_kernel_spmd
    nc = _get_nc()
    return run_bass_kernel_spmd(nc, in_maps, list(range(NCORES)), trace=trace)


def kernel(input_ids, embed, w_inner, w_head, phi, b,
           alpha_pre, alpha_post, alpha_res):
    in_maps = _make_in_maps(input_ids, embed, w_inner, w_head, phi, b,
                            alpha_pre, alpha_post, alpha_res)
    res = _run(in_maps).results
    out = np.concatenate([np.asarray(res[i]["out"]) for i in range(NCORES)],
                         axis=1)[:, :VOCAB]
    return np.ascontiguousarray(out.reshape(B, S, VOCAB).astype(np.float32))


# revision 30
# speedup vs baseline: 1.0266x; 1.0266x over previous
"""Trainium2 Bass kernel for nn_MiniMHCLM (moe_routing).

Strategy (8 NeuronCores, SPMD, no collectives):
  - vocab-sharded head matmul: core i holds w_head rows [i*VS:(i+1)*VS]
    (host-sliced, zero-padded to uniform VS) transposed to k-major bf16;
    it computes logits for all 4096 tokens x its vocab slice and the host
    concatenates along vocab.
  - token embeddings are pre-gathered AND pre-transposed on the host into
    xT [K, NT] bf16 (numerically identical to embed[ids].astype(bf16)),
    so the device pipeline runs fully k-major with zero PE transposes of x
    and no indirect DMA.
  - per-token coeffs: phi-stationary matmul gives logits^T [24, T] plus a
    ones-matmul row of sum(x^2); one small PE transpose per 128-token
    chunk moves both to token-major for the RMS scale, sigmoid/exp and
    the Sinkhorn iterations (DVE/ACT, batched per token group).
  - mixing runs transposed: per-token coeffs are PE-transposed back to
    [24, T], bounced through a DRAM scratch row and broadcast across
    partitions with a stride-0 DMA read; x_merge^T is then built with
    DVE/GPSIMD multiply-adds and fed straight into the head matmul as
    the stationary operand.
  - head matmul in bf16 with fp32 PSUM; PSUM evacuated by ACT copies to
    bf16 and DMA'd to DRAM bf16 (host converts to fp32).
  - software pipeline over token groups with a warm-up ramp
    (128/256/512... tokens): group g's head matmuls overlap group g+1's
    coeff pipeline and group g+2's logits, keeping the PE stream dense.
"""

import numpy as np

HC, C, TMAX = 4, 256, 8
RMS_EPS, PRE_EPS, SINK_EPS, POST_MULT = 1e-6, 1e-4, 1e-6, 2.0
VOCAB = 50257
B, S = 2, 2048
K = HC * C            # 1024
M = HC * HC + 2 * HC  # 24
NKC = K // 128        # 8 k-chunks
NCORES = 8
NT = B * S            # 4096
VS = 6283             # vocab rows per core (8*6283 = 50264 >= 50257)
VW = 512
NV = (VS + VW - 1) // VW          # 13 head tiles (12x512 + 139)
SCS = [128, 256] + [512] * 7 + [128]   # token-group ramp, sum = NT
OFF = [sum(SCS[:i]) for i in range(len(SCS))]
NG = len(SCS)
assert sum(SCS) == NT


def _build():
    from contextlib import ExitStack
    from concourse import bass, bacc, mybir
    import concourse.tile as tile
    from concourse.masks import make_identity

    f32 = mybir.dt.float32
    bf16 = mybir.dt.bfloat16
    AX = mybir.AxisListType
    OP = mybir.AluOpType
    AF = mybir.ActivationFunctionType

    nc = bacc.Bacc(target_bir_lowering=False)
    xt_p = nc.declare_dram_parameter("xt", [K, NT], bf16, False)
    wvt_p = nc.declare_dram_parameter("wvt", [K, VS], bf16, False)
    wit_p = nc.declare_dram_parameter("wit", [C, C], bf16, False)
    phi_p = nc.declare_dram_parameter("phi", [K, M], bf16, False)
    b_p = nc.declare_dram_parameter("b", [1, M], f32, False)
    al_p = nc.declare_dram_parameter("al", [1, 3], f32, False)
    out_p = nc.declare_dram_parameter("out", [NT, VS], bf16, True)

    with ExitStack() as ctx:
        tc = ctx.enter_context(tile.TileContext(nc))
        const = ctx.enter_context(tc.tile_pool(name="const", bufs=1))
        wtp = ctx.enter_context(tc.tile_pool(name="wtp", bufs=1))
        xtp = ctx.enter_context(tc.tile_pool(name="xtp", bufs=3))
        lgp = ctx.enter_context(tc.tile_pool(name="lgp", bufs=2))
        cfp = ctx.enter_context(tc.tile_pool(name="cfp", bufs=2))
        plp = ctx.enter_context(tc.tile_pool(name="plp", bufs=1))
        mxp = ctx.enter_context(tc.tile_pool(name="mxp", bufs=2))
        wkp = ctx.enter_context(tc.tile_pool(name="wkp", bufs=4))
        x2p = ctx.enter_context(tc.tile_pool(name="x2p", bufs=1))
        stp = ctx.enter_context(tc.tile_pool(name="stp", bufs=5))
        psh = ctx.enter_context(tc.tile_pool(name="psh", bufs=4, space="PSUM"))
        psa = ctx.enter_context(tc.tile_pool(name="psa", bufs=1, space="PSUM"))
        psb = ctx.enter_context(tc.tile_pool(name="psb", bufs=1, space="PSUM"))
        pst = ctx.enter_context(tc.tile_pool(name="pst", bufs=1, space="PSUM"))
        psf = ctx.enter_context(tc.tile_pool(name="psf", bufs=1, space="PSUM"))
        drp = ctx.enter_context(tc.tile_pool(name="drp", bufs=2, space="DRAM"))

        # ---------------- constants ----------------
        ident = const.tile([128, 128], bf16)
        make_identity(nc, ident[:])
        identf = const.tile([128, 128], f32)
        make_identity(nc, identf[:])

        cst = const.tile([128, 2], f32)
        nc.vector.memset(cst[:, 0:1], 0.0)
        nc.vector.memset(cst[:, 1:2], RMS_EPS)
        zero_b = cst[:, 0:1]
        eps_b = cst[:, 1:2]

        ones = const.tile([128, 1], bf16)
        nc.vector.memset(ones[:], 1.0)

        phi_sb = const.tile([128, NKC * M], bf16)
        for kc in range(NKC):
            nc.sync.dma_start(out=phi_sb[:, kc * M:(kc + 1) * M],
                              in_=phi_p[kc * 128:(kc + 1) * 128, :])
        b_bc = const.tile([128, M], f32)
        nc.sync.dma_start(out=b_bc[:], in_=b_p[0:1, :].to_broadcast([128, M]))
        al_bc = const.tile([128, 3], f32)
        nc.sync.dma_start(out=al_bc[:], in_=al_p[0:1, :].to_broadcast([128, 3]))

        # w_inner^T (k-major [c, o]) as 2 row bands
        wit_sb = const.tile([128, 2 * C], bf16)
        for h in range(2):
            nc.sync.dma_start(out=wit_sb[:, h * C:(h + 1) * C],
                              in_=wit_p[h * 128:(h + 1) * 128, :])

        # w_head^T slice, 8 k row bands
        wt_all = wtp.tile([128, NKC * VS], bf16, tag="wt_all")
        for kc in range(NKC):
            nc.sync.dma_start(out=wt_all[:, kc * VS:(kc + 1) * VS],
                              in_=wvt_p[kc * 128:(kc + 1) * 128, :])

        st = {}  # per-group live tiles

        # ---------------- pipeline stages ----------------
        def stage_lg(g):
            """xT DMA, phi logits^T + sumsq row, transpose to token-major,
            RMS scale + coeff activations + Sinkhorn -> coefs."""
            gt, t0 = SCS[g], OFF[g]
            nch = gt // 128
            xtg = xtp.tile([128, NKC * gt], bf16, tag="xtg", name=f"xtg{g}")
            for kc in range(NKC):
                nc.sync.dma_start(
                    out=xtg[:, kc * gt:(kc + 1) * gt],
                    in_=xt_p[kc * 128:(kc + 1) * 128, t0:t0 + gt])

            # squares for the RMS sum (bf16 is plenty for the mean)
            x2s = []
            for half in range(2):
                x2 = x2p.tile([128, 4 * gt], bf16, tag=f"x2{half}",
                              name=f"x2_{g}_{half}")
                sl = slice(half * 4 * gt, (half + 1) * 4 * gt)
                nc.gpsimd.tensor_tensor(
                    out=x2[:], in0=xtg[:, sl], in1=xtg[:, sl], op=OP.mult)
                x2s.append(x2)

            pslg = psa.tile([32, gt], f32, tag="pslg")
            psss = psb.tile([32, gt], f32, tag="psss")
            for kc in range(NKC):
                nc.tensor.matmul(
                    out=pslg[0:M, :],
                    lhsT=phi_sb[:, kc * M:(kc + 1) * M],
                    rhs=xtg[:, kc * gt:(kc + 1) * gt],
                    start=(kc == 0), stop=(kc == NKC - 1))
            for kc in range(NKC):
                nc.tensor.matmul(
                    out=psss[0:1, :],
                    lhsT=ones[:],
                    rhs=x2s[kc // 4][:, (kc % 4) * gt:(kc % 4 + 1) * gt],
                    start=(kc == 0), stop=(kc == NKC - 1))

            lgsb = lgp.tile([32, gt], f32, tag="lgsb", name=f"lgsb{g}")
            nc.scalar.copy(lgsb[0:M, :], pslg[0:M, :])
            ssq = lgp.tile([1, gt], f32, tag="ssq", name=f"ssq{g}")
            nc.scalar.copy(ssq[0:1, :], psss[0:1, :])

            # token-major [128, nch, 24] + per-token sumsq column
            lgtm = lgp.tile([128, nch * 32], f32, tag="lgtm", name=f"lgtm{g}")
            msq = lgp.tile([128, nch], f32, tag="msq", name=f"msq{g}")
            for tcx in range(nch):
                pT = pst.tile([128, 128], f32, tag="psT")
                nc.tensor.transpose(
                    out=pT[:, 0:M],
                    in_=lgsb[0:M, tcx * 128:(tcx + 1) * 128],
                    identity=identf[0:M, 0:M])
                nc.tensor.transpose(
                    out=pT[:, M:M + 1],
                    in_=ssq[0:1, tcx * 128:(tcx + 1) * 128],
                    identity=identf[0:1, 0:1])
                nc.scalar.copy(lgtm[:, tcx * 32:tcx * 32 + M], pT[:, 0:M])
                nc.scalar.copy(msq[:, tcx:tcx + 1], pT[:, M:M + 1])
            lgv = lgtm[:].rearrange("p (c w) -> p c w", w=32)

            # scl = 1/sqrt(mean+eps)
            scl = lgp.tile([128, nch], f32, tag="scl", name=f"scl{g}")
            nc.scalar.activation(out=scl[:], in_=msq[:],
                                 func=AF.Sqrt, scale=1.0 / K, bias=eps_b)
            nc.vector.reciprocal(scl[:], scl[:])
            for tcx in range(nch):
                nc.vector.tensor_scalar_mul(
                    lgv[:, tcx, 0:M], lgv[:, tcx, 0:M], scl[:, tcx:tcx + 1])
            nc.vector.tensor_tensor(
                out=lgv[:, :, 0:M], in0=lgv[:, :, 0:M],
                in1=b_bc[:][:, None, :].to_broadcast([128, nch, M]), op=OP.add)

            # coefs [128, nch, 24]: [0:16]=exp(res), [16:20]=h_pre,
            # [20:24]=h_post2
            coefs = cfp.tile([128, nch * M], f32, tag="coefs",
                             name=f"coefs{g}")
            cfv = coefs[:].rearrange("p (c m) -> p c m", m=M)
            nc.scalar.activation(out=cfv[:, :, 16:20], in_=lgv[:, :, 0:4],
                                 func=AF.Sigmoid, bias=zero_b,
                                 scale=al_bc[:, 0:1])
            nc.vector.tensor_scalar_add(cfv[:, :, 16:20], cfv[:, :, 16:20],
                                        PRE_EPS)
            nc.scalar.activation(out=cfv[:, :, 20:24], in_=lgv[:, :, 4:8],
                                 func=AF.Sigmoid, bias=zero_b,
                                 scale=al_bc[:, 1:2])
            nc.vector.tensor_scalar_mul(cfv[:, :, 20:24], cfv[:, :, 20:24],
                                        POST_MULT)
            nc.scalar.activation(out=cfv[:, :, 0:16], in_=lgv[:, :, 8:24],
                                 func=AF.Exp, bias=zero_b, scale=al_bc[:, 2:3])

            # batched Sinkhorn on cfv[:, :, 0:16].
            # SINK_EPS (1e-6 vs O(1) row sums) is dropped: it shifts the
            # result by ~1e-6 relative, far below the bf16 noise floor.
            mv4 = cfv[:, :, 0:16].rearrange("p c (o i) -> p c o i", i=4)
            mv4t = cfv[:, :, 0:16].rearrange("p c (o i) -> p c i o", i=4)
            for _ in range(TMAX):
                rs = wkp.tile([128, 4 * 4], f32, tag="rs")
                rsv = rs[:, 0:nch * 4].rearrange("p (c o) -> p c o", c=nch)
                nc.vector.tensor_reduce(rsv, mv4, axis=AX.X, op=OP.add)
                nc.vector.reciprocal(rs[:, 0:nch * 4], rs[:, 0:nch * 4])
                nc.vector.tensor_tensor(
                    out=mv4, in0=mv4,
                    in1=rsv[:, :, :, None].to_broadcast([128, nch, 4, 4]),
                    op=OP.mult)
                cs = wkp.tile([128, 4 * 4], f32, tag="cs")
                csv = cs[:, 0:nch * 4].rearrange("p (c i) -> p c i", c=nch)
                nc.vector.tensor_reduce(csv, mv4t, axis=AX.X, op=OP.add)
                nc.vector.reciprocal(cs[:, 0:nch * 4], cs[:, 0:nch * 4])
                nc.vector.tensor_tensor(
                    out=mv4, in0=mv4,
                    in1=csv[:, :, None, :].to_broadcast([128, nch, 4, 4]),
                    op=OP.mult)
            st[g] = dict(xtg=xtg, coefs=coefs)

        def stage_planes(g):
            """Transpose coefs back to [24, T]; bounce through DRAM and
            broadcast-read -> planes [128, 24*gt]."""
            gt = SCS[g]
            nch = gt // 128
            coefs = st[g]["coefs"]
            ctstg = cfp.tile([32, gt], bf16, tag="ctstg", name=f"ctstg{g}")
            for tcx in range(nch):
                pT = pst.tile([128, 128], f32, tag="psT")
                nc.tensor.transpose(
                    out=pT[0:M, 0:128],
                    in_=coefs[:, tcx * M:(tcx + 1) * M],
                    identity=identf[:, 0:128])
                nc.scalar.copy(
                    ctstg[0:M, tcx * 128:(tcx + 1) * 128], pT[0:M, 0:128])
            dtile = drp.tile([1, M * gt], bf16, tag="cfdram",
                             name=f"cfdram{g}")
            nc.sync.dma_start(
                out=dtile[0:1, :].rearrange("x (c t) -> (x c) t", c=M),
                in_=ctstg[0:M, :])
            planes = plp.tile([128, M * gt], bf16, tag="planes",
                              name=f"planes{g}")
            nc.sync.dma_start(
                out=planes[:],
                in_=dtile[0:1, :].to_broadcast([128, M * gt]))
            st[g]["planes"] = planes
            # x_in^T = sum_i h_pre[i] * x^T[i]  (2 half-chunks of c)
            xtg = st[g]["xtg"]
            xin = mxp.tile([128, 2 * gt], bf16, tag="xin", name=f"xin{g}")
            for h in range(2):
                seg = xin[:, h * gt:(h + 1) * gt]
                nc.vector.tensor_tensor(
                    out=seg, in0=xtg[:, h * gt:(h + 1) * gt],
                    in1=planes[:, 16 * gt:17 * gt], op=OP.mult)
                for i in range(1, HC):
                    tmp = wkp.tile([128, 512], bf16, tag="tmp")
                    nc.vector.tensor_tensor(
                        out=tmp[:, 0:gt],
                        in0=xtg[:, (i * 2 + h) * gt:(i * 2 + h + 1) * gt],
                        in1=planes[:, (16 + i) * gt:(17 + i) * gt],
                        op=OP.mult)
                    eng = nc.vector if i % 2 else nc.gpsimd
                    eng.tensor_add(seg, seg, tmp[:, 0:gt])
            st[g]["xin"] = xin

        def stage_fo(g):
            """f_out^T = w_inner @ x_in^T : 2 o-blocks x 2 c-halves."""
            gt = SCS[g]
            xin = st[g]["xin"]
            fo = mxp.tile([128, 2 * gt], bf16, tag="fo", name=f"fo{g}")
            for ob in range(2):
                pf = psf.tile([128, gt], f32, tag="psf")
                for h in range(2):
                    nc.tensor.matmul(
                        out=pf[:],
                        lhsT=wit_sb[:, h * C + ob * 128:h * C + (ob + 1) * 128],
                        rhs=xin[:, h * gt:(h + 1) * gt],
                        start=(h == 0), stop=(h == 1))
                nc.scalar.copy(fo[:, ob * gt:(ob + 1) * gt], pf[:])
            st[g]["fo"] = fo

        def stage_mix(g):
            """x_merge^T[kc] = sum_i res[o,i]*x^T[i,h] + post2[o]*f_out^T[h]"""
            gt = SCS[g]
            xtg, planes, fo = st[g]["xtg"], st[g]["planes"], st[g]["fo"]
            xmg = mxp.tile([128, NKC * gt], bf16, tag="xmg", name=f"xmg{g}")
            for kc in range(NKC):
                o, h = kc // 2, kc % 2
                seg = xmg[:, kc * gt:(kc + 1) * gt]
                nc.vector.tensor_tensor(
                    out=seg, in0=xtg[:, h * gt:(h + 1) * gt],
                    in1=planes[:, (o * 4) * gt:(o * 4 + 1) * gt], op=OP.mult)
                for i in range(1, HC):
                    tmp = wkp.tile([128, 512], bf16, tag="tmp")
                    nc.vector.tensor_tensor(
                        out=tmp[:, 0:gt],
                        in0=xtg[:, (i * 2 + h) * gt:(i * 2 + h + 1) * gt],
                        in1=planes[:, (o * 4 + i) * gt:(o * 4 + i + 1) * gt],
                        op=OP.mult)
                    eng = nc.vector if i % 2 else nc.gpsimd
                    eng.tensor_add(seg, seg, tmp[:, 0:gt])
                tmp = wkp.tile([128, 512], bf16, tag="tmp")
                nc.vector.tensor_tensor(
                    out=tmp[:, 0:gt], in0=fo[:, h * gt:(h + 1) * gt],
                    in1=planes[:, (20 + o) * gt:(21 + o) * gt], op=OP.mult)
                nc.gpsimd.tensor_add(seg, seg, tmp[:, 0:gt])
            st[g]["xmg"] = xmg

        def head_chunk(g, tcx):
            gt = SCS[g]
            xmg = st[g]["xmg"]
            t0 = OFF[g] + tcx * 128
            stg = None
            for v in range(NV):
                w = min(VW, VS - v * VW)
                ph = psh.tile([128, VW], f32, tag="psh")
                for kc in range(NKC):
                    nc.tensor.matmul(
                        out=ph[:, 0:w],
                        lhsT=xmg[:, kc * gt + tcx * 128:
                                 kc * gt + (tcx + 1) * 128],
                        rhs=wt_all[:, kc * VS + v * VW:kc * VS + v * VW + w],
                        start=(kc == 0), stop=(kc == NKC - 1))
                # pair two v-tiles per staging tile / output DMA
                half = v % 2
                if half == 0:
                    stg = stp.tile([128, 2 * VW], bf16, tag="stg")
                nc.scalar.copy(stg[:, half * VW:half * VW + w], ph[:, 0:w])
                if half == 1 or v == NV - 1:
                    v0 = v - half
                    ww = min(2 * VW, VS - v0 * VW)
                    nc.sync.dma_start(
                        out=out_p[t0:t0 + 128, v0 * VW:v0 * VW + ww],
                        in_=stg[:, 0:ww])

        # ---------------- emission (software pipeline) ----------------
        stage_lg(0)
        stage_lg(1)
        stage_planes(0)
        stage_fo(0)
        stage_mix(0)
        for g in range(NG):
            nch = SCS[g] // 128
            inserts = {0: [], 1: [], 2: []}
            if g + 1 < NG:
                inserts[0].append(("planes", g + 1))
                inserts[min(1, nch - 1)].append(("fo", g + 1))
                inserts[min(1, nch - 1)].append(("mix", g + 1))
            if g + 2 < NG:
                inserts[min(2, nch - 1)].append(("lg", g + 2))
            for tcx in range(nch):
                head_chunk(g, tcx)
                for kind, gg in inserts.get(tcx, []):
                    if kind == "planes":
                        stage_planes(gg)
                    elif kind == "fo":
                        stage_fo(gg)
                    elif kind == "mix":
                        stage_mix(gg)
                    else:
                        stage_lg(gg)
            del st[g]

    if not nc.is_finalized():
        nc.finalize()
    return nc


_NC_CACHE = {}


def _get_nc():
    if "nc" not in _NC_CACHE:
        _NC_CACHE["nc"] = _build()
    return _NC_CACHE["nc"]


def _make_in_maps(input_ids, embed, w_inner, w_head, phi, b,
                  alpha_pre, alpha_post, alpha_res):
    import ml_dtypes
    bf = ml_dtypes.bfloat16

    ids = np.asarray(input_ids).reshape(-1).astype(np.int64)
    x = np.asarray(embed)[ids].astype(bf)                 # [NT, K]
    xt = np.ascontiguousarray(x.T)                        # [K, NT]
    phi_np = np.ascontiguousarray(np.asarray(phi).astype(bf))
    wit = np.ascontiguousarray(np.asarray(w_inner).astype(bf).T)  # [c, o]
    b_np = np.ascontiguousarray(np.asarray(b, dtype=np.float32).reshape(1, M))
    al = np.array([[np.asarray(alpha_pre).reshape(-1)[0],
                    np.asarray(alpha_post).reshape(-1)[0],
                    np.asarray(alpha_res).reshape(-1)[0]]], dtype=np.float32)
    wh = np.asarray(w_head).astype(bf)                    # [VOCAB, K]

    in_maps = []
    for i in range(NCORES):
        sl = wh[i * VS:(i + 1) * VS]                      # [<=VS, K]
        wvt = np.zeros((K, VS), bf)
        wvt[:, :sl.shape[0]] = sl.T
        in_maps.append(dict(xt=xt, wvt=np.ascontiguousarray(wvt),
                            wit=wit, phi=phi_np, b=b_np, al=al))
    return in_maps


def _run(in_maps, trace=False):
    from concourse.bass_utils import run_bass_kernel_spmd
    nc = _get_nc()
    return run_bass_kernel_spmd(nc, in_maps, list(range(NCORES)), trace=trace)


def kernel(input_ids, embed, w_inner, w_head, phi, b,
           alpha_pre, alpha_post, alpha_res):
    in_maps = _make_in_maps(input_ids, embed, w_inner, w_head, phi, b,
                            alpha_pre, alpha_post, alpha_res)
    res = _run(in_maps).results
    out = np.concatenate([np.asarray(res[i]["out"]) for i in range(NCORES)],
                         axis=1)[:, :VOCAB]
    return np.ascontiguousarray(out.reshape(B, S, VOCAB).astype(np.float32))


# revision 31
# speedup vs baseline: 1.0371x; 1.0102x over previous
"""Trainium2 Bass kernel for nn_MiniMHCLM (moe_routing).

Strategy (8 NeuronCores, SPMD, no collectives):
  - vocab-sharded head matmul: core i holds w_head rows [i*VS:(i+1)*VS]
    (host-sliced, zero-padded to uniform VS) transposed to k-major bf16;
    it computes logits for all 4096 tokens x its vocab slice and the host
    concatenates along vocab.
  - token embeddings are pre-gathered AND pre-transposed on the host into
    xT [K, NT] bf16 (numerically identical to embed[ids].astype(bf16)),
    so the device pipeline runs fully k-major with zero PE transposes of x
    and no indirect DMA.
  - per-token coeffs: phi-stationary matmul gives logits^T [24, T] plus a
    ones-matmul row of sum(x^2); one small PE transpose per 128-token
    chunk moves both to token-major for the RMS scale, sigmoid/exp and
    the Sinkhorn iterations (DVE/ACT, batched per token group).
  - mixing runs transposed: per-token coeffs are PE-transposed back to
    [24, T], bounced through a DRAM scratch row and broadcast across
    partitions with a stride-0 DMA read; x_merge^T is then built with
    DVE/GPSIMD multiply-adds and fed straight into the head matmul as
    the stationary operand.
  - head matmul in bf16 with fp32 PSUM; PSUM evacuated by ACT copies to
    bf16 and DMA'd to DRAM bf16 (host converts to fp32).
  - software pipeline over token groups with a warm-up ramp
    (128/256/512... tokens): group g's head matmuls overlap group g+1's
    coeff pipeline and group g+2's logits, keeping the PE stream dense.
"""

import numpy as np

HC, C, TMAX = 4, 256, 8
RMS_EPS, PRE_EPS, SINK_EPS, POST_MULT = 1e-6, 1e-4, 1e-6, 2.0
VOCAB = 50257
B, S = 2, 2048
K = HC * C            # 1024
M = HC * HC + 2 * HC  # 24
NKC = K // 128        # 8 k-chunks
NCORES = 8
NT = B * S            # 4096
VS = 6283             # vocab rows per core (8*6283 = 50264 >= 50257)
VW = 512
NV = (VS + VW - 1) // VW          # 13 head tiles (12x512 + 139)
SCS = [128, 256] + [512] * 7 + [128]   # token-group ramp, sum = NT
OFF = [sum(SCS[:i]) for i in range(len(SCS))]
NG = len(SCS)
assert sum(SCS) == NT


def _build():
    from contextlib import ExitStack
    from concourse import bass, bacc, mybir
    import concourse.tile as tile
    from concourse.masks import make_identity

    f32 = mybir.dt.float32
    bf16 = mybir.dt.bfloat16
    AX = mybir.AxisListType
    OP = mybir.AluOpType
    AF = mybir.ActivationFunctionType

    nc = bacc.Bacc(target_bir_lowering=False)
    xt_p = nc.declare_dram_parameter("xt", [K, NT], bf16, False)
    wvt_p = nc.declare_dram_parameter("wvt", [K, VS], bf16, False)
    wit_p = nc.declare_dram_parameter("wit", [C, C], bf16, False)
    phi_p = nc.declare_dram_parameter("phi", [K, M], bf16, False)
    b_p = nc.declare_dram_parameter("b", [1, M], f32, False)
    al_p = nc.declare_dram_parameter("al", [1, 3], f32, False)
    out_p = nc.declare_dram_parameter("out", [NT, VS], bf16, True)

    with ExitStack() as ctx:
        tc = ctx.enter_context(tile.TileContext(nc))
        const = ctx.enter_context(tc.tile_pool(name="const", bufs=1))
        wtp = ctx.enter_context(tc.tile_pool(name="wtp", bufs=1))
        xtp = ctx.enter_context(tc.tile_pool(name="xtp", bufs=3))
        lgp = ctx.enter_context(tc.tile_pool(name="lgp", bufs=2))
        cfp = ctx.enter_context(tc.tile_pool(name="cfp", bufs=2))
        plp = ctx.enter_context(tc.tile_pool(name="plp", bufs=1))
        mxp = ctx.enter_context(tc.tile_pool(name="mxp", bufs=2))
        wkp = ctx.enter_context(tc.tile_pool(name="wkp", bufs=4))
        x2p = ctx.enter_context(tc.tile_pool(name="x2p", bufs=1))
        stp = ctx.enter_context(tc.tile_pool(name="stp", bufs=5))
        psh = ctx.enter_context(tc.tile_pool(name="psh", bufs=4, space="PSUM"))
        psa = ctx.enter_context(tc.tile_pool(name="psa", bufs=1, space="PSUM"))
        psb = ctx.enter_context(tc.tile_pool(name="psb", bufs=1, space="PSUM"))
        pst = ctx.enter_context(tc.tile_pool(name="pst", bufs=1, space="PSUM"))
        psf = ctx.enter_context(tc.tile_pool(name="psf", bufs=1, space="PSUM"))
        drp = ctx.enter_context(tc.tile_pool(name="drp", bufs=2, space="DRAM"))

        # ---------------- constants ----------------
        ident = const.tile([128, 128], bf16)
        make_identity(nc, ident[:])
        identf = const.tile([128, 128], f32)
        make_identity(nc, identf[:])

        cst = const.tile([128, 2], f32)
        nc.vector.memset(cst[:, 0:1], 0.0)
        nc.vector.memset(cst[:, 1:2], RMS_EPS)
        zero_b = cst[:, 0:1]
        eps_b = cst[:, 1:2]

        ones = const.tile([128, 1], bf16)
        nc.vector.memset(ones[:], 1.0)

        phi_sb = const.tile([128, NKC * M], bf16)
        for kc in range(NKC):
            nc.sync.dma_start(out=phi_sb[:, kc * M:(kc + 1) * M],
                              in_=phi_p[kc * 128:(kc + 1) * 128, :])
        b_bc = const.tile([128, M], f32)
        nc.sync.dma_start(out=b_bc[:], in_=b_p[0:1, :].to_broadcast([128, M]))
        al_bc = const.tile([128, 3], f32)
        nc.sync.dma_start(out=al_bc[:], in_=al_p[0:1, :].to_broadcast([128, 3]))

        # w_inner^T (k-major [c, o]) as 2 row bands
        wit_sb = const.tile([128, 2 * C], bf16)
        for h in range(2):
            nc.sync.dma_start(out=wit_sb[:, h * C:(h + 1) * C],
                              in_=wit_p[h * 128:(h + 1) * 128, :])

        # w_head^T slice, 8 k row bands. Issued on the ACT HWDGE queue so
        # the 13.6MB load doesn't serialize ahead of the first groups' xT
        # DMAs on the sync queue.
        wt_all = wtp.tile([128, NKC * VS], bf16, tag="wt_all")
        for kc in range(NKC):
            nc.scalar.dma_start(out=wt_all[:, kc * VS:(kc + 1) * VS],
                                in_=wvt_p[kc * 128:(kc + 1) * 128, :])

        st = {}  # per-group live tiles

        # ---------------- pipeline stages ----------------
        def stage_lg(g):
            """xT DMA, phi logits^T + sumsq row, transpose to token-major,
            RMS scale + coeff activations + Sinkhorn -> coefs."""
            gt, t0 = SCS[g], OFF[g]
            nch = gt // 128
            xtg = xtp.tile([128, NKC * gt], bf16, tag="xtg", name=f"xtg{g}")
            for kc in range(NKC):
                nc.sync.dma_start(
                    out=xtg[:, kc * gt:(kc + 1) * gt],
                    in_=xt_p[kc * 128:(kc + 1) * 128, t0:t0 + gt])

            # squares for the RMS sum (bf16 is plenty for the mean)
            x2s = []
            for half in range(2):
                x2 = x2p.tile([128, 4 * gt], bf16, tag=f"x2{half}",
                              name=f"x2_{g}_{half}")
                sl = slice(half * 4 * gt, (half + 1) * 4 * gt)
                nc.gpsimd.tensor_tensor(
                    out=x2[:], in0=xtg[:, sl], in1=xtg[:, sl], op=OP.mult)
                x2s.append(x2)

            pslg = psa.tile([32, gt], f32, tag="pslg")
            psss = psb.tile([32, gt], f32, tag="psss")
            for kc in range(NKC):
                nc.tensor.matmul(
                    out=pslg[0:M, :],
                    lhsT=phi_sb[:, kc * M:(kc + 1) * M],
                    rhs=xtg[:, kc * gt:(kc + 1) * gt],
                    start=(kc == 0), stop=(kc == NKC - 1))
            for kc in range(NKC):
                nc.tensor.matmul(
                    out=psss[0:1, :],
                    lhsT=ones[:],
                    rhs=x2s[kc // 4][:, (kc % 4) * gt:(kc % 4 + 1) * gt],
                    start=(kc == 0), stop=(kc == NKC - 1))

            lgsb = lgp.tile([32, gt], f32, tag="lgsb", name=f"lgsb{g}")
            nc.scalar.copy(lgsb[0:M, :], pslg[0:M, :])
            ssq = lgp.tile([1, gt], f32, tag="ssq", name=f"ssq{g}")
            nc.scalar.copy(ssq[0:1, :], psss[0:1, :])

            # token-major [128, nch, 24] + per-token sumsq column
            lgtm = lgp.tile([128, nch * 32], f32, tag="lgtm", name=f"lgtm{g}")
            msq = lgp.tile([128, nch], f32, tag="msq", name=f"msq{g}")
            for tcx in range(nch):
                pT = pst.tile([128, 128], f32, tag="psT")
                nc.tensor.transpose(
                    out=pT[:, 0:M],
                    in_=lgsb[0:M, tcx * 128:(tcx + 1) * 128],
                    identity=identf[0:M, 0:M])
                nc.tensor.transpose(
                    out=pT[:, M:M + 1],
                    in_=ssq[0:1, tcx * 128:(tcx + 1) * 128],
                    identity=identf[0:1, 0:1])
                nc.scalar.copy(lgtm[:, tcx * 32:tcx * 32 + M], pT[:, 0:M])
                nc.scalar.copy(msq[:, tcx:tcx + 1], pT[:, M:M + 1])
            lgv = lgtm[:].rearrange("p (c w) -> p c w", w=32)

            # scl = 1/sqrt(mean+eps)
            scl = lgp.tile([128, nch], f32, tag="scl", name=f"scl{g}")
            nc.scalar.activation(out=scl[:], in_=msq[:],
                                 func=AF.Sqrt, scale=1.0 / K, bias=eps_b)
            nc.vector.reciprocal(scl[:], scl[:])
            for tcx in range(nch):
                nc.vector.tensor_scalar_mul(
                    lgv[:, tcx, 0:M], lgv[:, tcx, 0:M], scl[:, tcx:tcx + 1])
            nc.vector.tensor_tensor(
                out=lgv[:, :, 0:M], in0=lgv[:, :, 0:M],
                in1=b_bc[:][:, None, :].to_broadcast([128, nch, M]), op=OP.add)

            # coefs [128, nch, 24]: [0:16]=exp(res), [16:20]=h_pre,
            # [20:24]=h_post2
            coefs = cfp.tile([128, nch * M], f32, tag="coefs",
                             name=f"coefs{g}")
            cfv = coefs[:].rearrange("p (c m) -> p c m", m=M)
            nc.scalar.activation(out=cfv[:, :, 16:20], in_=lgv[:, :, 0:4],
                                 func=AF.Sigmoid, bias=zero_b,
                                 scale=al_bc[:, 0:1])
            nc.vector.tensor_scalar_add(cfv[:, :, 16:20], cfv[:, :, 16:20],
                                        PRE_EPS)
            nc.scalar.activation(out=cfv[:, :, 20:24], in_=lgv[:, :, 4:8],
                                 func=AF.Sigmoid, bias=zero_b,
                                 scale=al_bc[:, 1:2])
            nc.vector.tensor_scalar_mul(cfv[:, :, 20:24], cfv[:, :, 20:24],
                                        POST_MULT)
            nc.scalar.activation(out=cfv[:, :, 0:16], in_=lgv[:, :, 8:24],
                                 func=AF.Exp, bias=zero_b, scale=al_bc[:, 2:3])

            # batched Sinkhorn on cfv[:, :, 0:16].
            # SINK_EPS (1e-6 vs O(1) row sums) is dropped: it shifts the
            # result by ~1e-6 relative, far below the bf16 noise floor.
            mv4 = cfv[:, :, 0:16].rearrange("p c (o i) -> p c o i", i=4)
            mv4t = cfv[:, :, 0:16].rearrange("p c (o i) -> p c i o", i=4)
            for _ in range(TMAX):
                rs = wkp.tile([128, 4 * 4], f32, tag="rs")
                rsv = rs[:, 0:nch * 4].rearrange("p (c o) -> p c o", c=nch)
                nc.vector.tensor_reduce(rsv, mv4, axis=AX.X, op=OP.add)
                nc.vector.reciprocal(rs[:, 0:nch * 4], rs[:, 0:nch * 4])
                nc.vector.tensor_tensor(
                    out=mv4, in0=mv4,
                    in1=rsv[:, :, :, None].to_broadcast([128, nch, 4, 4]),
                    op=OP.mult)
                cs = wkp.tile([128, 4 * 4], f32, tag="cs")
                csv = cs[:, 0:nch * 4].rearrange("p (c i) -> p c i", c=nch)
                nc.vector.tensor_reduce(csv, mv4t, axis=AX.X, op=OP.add)
                nc.vector.reciprocal(cs[:, 0:nch * 4], cs[:, 0:nch * 4])
                nc.vector.tensor_tensor(
                    out=mv4, in0=mv4,
                    in1=csv[:, :, None, :].to_broadcast([128, nch, 4, 4]),
                    op=OP.mult)
            st[g] = dict(xtg=xtg, coefs=coefs)

        def stage_planes(g):
            """Transpose coefs back to [24, T]; bounce through DRAM and
            broadcast-read -> planes [128, 24*gt]."""
            gt = SCS[g]
            nch = gt // 128
            coefs = st[g]["coefs"]
            ctstg = cfp.tile([32, gt], bf16, tag="ctstg", name=f"ctstg{g}")
            for tcx in range(nch):
                pT = pst.tile([128, 128], f32, tag="psT")
                nc.tensor.transpose(
                    out=pT[0:M, 0:128],
                    in_=coefs[:, tcx * M:(tcx + 1) * M],
                    identity=identf[:, 0:128])
                nc.scalar.copy(
                    ctstg[0:M, tcx * 128:(tcx + 1) * 128], pT[0:M, 0:128])
            dtile = drp.tile([1, M * gt], bf16, tag="cfdram",
                             name=f"cfdram{g}")
            nc.sync.dma_start(
                out=dtile[0:1, :].rearrange("x (c t) -> (x c) t", c=M),
                in_=ctstg[0:M, :])
            planes = plp.tile([128, M * gt], bf16, tag="planes",
                              name=f"planes{g}")
            nc.sync.dma_start(
                out=planes[:],
                in_=dtile[0:1, :].to_broadcast([128, M * gt]))
            st[g]["planes"] = planes
            # x_in^T = sum_i h_pre[i] * x^T[i]  (2 half-chunks of c)
            xtg = st[g]["xtg"]
            xin = mxp.tile([128, 2 * gt], bf16, tag="xin", name=f"xin{g}")
            for h in range(2):
                seg = xin[:, h * gt:(h + 1) * gt]
                nc.vector.tensor_tensor(
                    out=seg, in0=xtg[:, h * gt:(h + 1) * gt],
                    in1=planes[:, 16 * gt:17 * gt], op=OP.mult)
                for i in range(1, HC):
                    tmp = wkp.tile([128, 512], bf16, tag="tmp")
                    nc.vector.tensor_tensor(
                        out=tmp[:, 0:gt],
                        in0=xtg[:, (i * 2 + h) * gt:(i * 2 + h + 1) * gt],
                        in1=planes[:, (16 + i) * gt:(17 + i) * gt],
                        op=OP.mult)
                    eng = nc.vector if i % 2 else nc.gpsimd
                    eng.tensor_add(seg, seg, tmp[:, 0:gt])
            st[g]["xin"] = xin

        def stage_fo(g):
            """f_out^T = w_inner @ x_in^T : 2 o-blocks x 2 c-halves."""
            gt = SCS[g]
            xin = st[g]["xin"]
            fo = mxp.tile([128, 2 * gt], bf16, tag="fo", name=f"fo{g}")
            for ob in range(2):
                pf = psf.tile([128, gt], f32, tag="psf")
                for h in range(2):
                    nc.tensor.matmul(
                        out=pf[:],
                        lhsT=wit_sb[:, h * C + ob * 128:h * C + (ob + 1) * 128],
                        rhs=xin[:, h * gt:(h + 1) * gt],
                        start=(h == 0), stop=(h == 1))
                nc.scalar.copy(fo[:, ob * gt:(ob + 1) * gt], pf[:])
            st[g]["fo"] = fo

        def stage_mix(g):
            """x_merge^T[kc] = sum_i res[o,i]*x^T[i,h] + post2[o]*f_out^T[h]"""
            gt = SCS[g]
            xtg, planes, fo = st[g]["xtg"], st[g]["planes"], st[g]["fo"]
            xmg = mxp.tile([128, NKC * gt], bf16, tag="xmg", name=f"xmg{g}")
            for kc in range(NKC):
                o, h = kc // 2, kc % 2
                seg = xmg[:, kc * gt:(kc + 1) * gt]
                nc.vector.tensor_tensor(
                    out=seg, in0=xtg[:, h * gt:(h + 1) * gt],
                    in1=planes[:, (o * 4) * gt:(o * 4 + 1) * gt], op=OP.mult)
                for i in range(1, HC):
                    tmp = wkp.tile([128, 512], bf16, tag="tmp")
                    nc.vector.tensor_tensor(
                        out=tmp[:, 0:gt],
                        in0=xtg[:, (i * 2 + h) * gt:(i * 2 + h + 1) * gt],
                        in1=planes[:, (o * 4 + i) * gt:(o * 4 + i + 1) * gt],
                        op=OP.mult)
                    eng = nc.vector if i % 2 else nc.gpsimd
                    eng.tensor_add(seg, seg, tmp[:, 0:gt])
                tmp = wkp.tile([128, 512], bf16, tag="tmp")
                nc.vector.tensor_tensor(
                    out=tmp[:, 0:gt], in0=fo[:, h * gt:(h + 1) * gt],
                    in1=planes[:, (20 + o) * gt:(21 + o) * gt], op=OP.mult)
                nc.gpsimd.tensor_add(seg, seg, tmp[:, 0:gt])
            st[g]["xmg"] = xmg

        def head_chunk(g, tcx):
            gt = SCS[g]
            xmg = st[g]["xmg"]
            t0 = OFF[g] + tcx * 128
            stg = None
            for v in range(NV):
                w = min(VW, VS - v * VW)
                ph = psh.tile([128, VW], f32, tag="psh")
                for kc in range(NKC):
                    nc.tensor.matmul(
                        out=ph[:, 0:w],
                        lhsT=xmg[:, kc * gt + tcx * 128:
                                 kc * gt + (tcx + 1) * 128],
                        rhs=wt_all[:, kc * VS + v * VW:kc * VS + v * VW + w],
                        start=(kc == 0), stop=(kc == NKC - 1))
                # pair two v-tiles per staging tile / output DMA
                half = v % 2
                if half == 0:
                    stg = stp.tile([128, 2 * VW], bf16, tag="stg")
                nc.scalar.copy(stg[:, half * VW:half * VW + w], ph[:, 0:w])
                if half == 1 or v == NV - 1:
                    v0 = v - half
                    ww = min(2 * VW, VS - v0 * VW)
                    nc.sync.dma_start(
                        out=out_p[t0:t0 + 128, v0 * VW:v0 * VW + ww],
                        in_=stg[:, 0:ww])

        # ---------------- emission (software pipeline) ----------------
        stage_lg(0)
        stage_lg(1)
        stage_planes(0)
        stage_fo(0)
        stage_mix(0)
        for g in range(NG):
            nch = SCS[g] // 128
            inserts = {0: [], 1: [], 2: []}
            if g + 1 < NG:
                inserts[0].append(("planes", g + 1))
                inserts[min(1, nch - 1)].append(("fo", g + 1))
                inserts[min(1, nch - 1)].append(("mix", g + 1))
            if g + 2 < NG:
                inserts[min(2, nch - 1)].append(("lg", g + 2))
            for tcx in range(nch):
                head_chunk(g, tcx)
                for kind, gg in inserts.get(tcx, []):
                    if kind == "planes":
                        stage_planes(gg)
                    elif kind == "fo":
                        stage_fo(gg)
                    elif kind == "mix":
                        stage_mix(gg)
                    else:
                        stage_lg(gg)
            del st[g]

    if not nc.is_finalized():
        nc.finalize()
    return nc


_NC_CACHE = {}


def _get_nc():
    if "nc" not in _NC_CACHE:
        _NC_CACHE["nc"] = _build()
    return _NC_CACHE["nc"]


def _make_in_maps(input_ids, embed, w_inner, w_head, phi, b,
                  alpha_pre, alpha_post, alpha_res):
    import ml_dtypes
    bf = ml_dtypes.bfloat16

    ids = np.asarray(input_ids).reshape(-1).astype(np.int64)
    x = np.asarray(embed)[ids].astype(bf)                 # [NT, K]
    xt = np.ascontiguousarray(x.T)                        # [K, NT]
    phi_np = np.ascontiguousarray(np.asarray(phi).astype(bf))
    wit = np.ascontiguousarray(np.asarray(w_inner).astype(bf).T)  # [c, o]
    b_np = np.ascontiguousarray(np.asarray(b, dtype=np.float32).reshape(1, M))
    al = np.array([[np.asarray(alpha_pre).reshape(-1)[0],
                    np.asarray(alpha_post).reshape(-1)[0],
                    np.asarray(alpha_res).reshape(-1)[0]]], dtype=np.float32)
    wh = np.asarray(w_head).astype(bf)                    # [VOCAB, K]

    in_maps = []
    for i in range(NCORES):
        sl = wh[i * VS:(i + 1) * VS]                      # [<=VS, K]
        wvt = np.zeros((K, VS), bf)
        wvt[:, :sl.shape[0]] = sl.T
        in_maps.append(dict(xt=xt, wvt=np.ascontiguousarray(wvt),
                            wit=wit, phi=phi_np, b=b_np, al=al))
    return in_maps


def _run(in_maps, trace=False):
    from concourse.bass_utils import run_bass_kernel_spmd
    nc = _get_nc()
    return run_bass_kernel_spmd(nc, in_maps, list(range(NCORES)), trace=trace)


def kernel(input_ids, embed, w_inner, w_head, phi, b,
           alpha_pre, alpha_post, alpha_res):
    in_maps = _make_in_maps(input_ids, embed, w_inner, w_head, phi, b,
                            alpha_pre, alpha_post, alpha_res)
    res = _run(in_maps).results
    out = np.concatenate([np.asarray(res[i]["out"]) for i in range(NCORES)],
                         axis=1)[:, :VOCAB]
    return np.ascontiguousarray(out.reshape(B, S, VOCAB).astype(np.float32))


# revision 34
# speedup vs baseline: 1.0371x; 1.0000x over previous
"""Trainium2 Bass kernel for nn_MiniMHCLM (moe_routing).

Strategy (8 NeuronCores, SPMD, no collectives):
  - vocab-sharded head matmul: core i holds w_head rows [i*VS:(i+1)*VS]
    (host-sliced, zero-padded to uniform VS) transposed to k-major bf16;
    it computes logits for all 4096 tokens x its vocab slice and the host
    concatenates along vocab.
  - token embeddings are pre-gathered AND pre-transposed on the host into
    xT [K, NT] bf16 (numerically identical to embed[ids].astype(bf16)),
    so the device pipeline runs fully k-major with zero PE transposes of x
    and no indirect DMA.
  - per-token coeffs: phi-stationary matmul gives logits^T [24, T] plus a
    ones-matmul row of sum(x^2); one small PE transpose per 128-token
    chunk moves both to token-major for the RMS scale, sigmoid/exp and
    the Sinkhorn iterations (DVE/ACT, batched per token group).
  - mixing runs transposed: per-token coeffs are PE-transposed back to
    [24, T], bounced through a DRAM scratch row and broadcast across
    partitions with a stride-0 DMA read; x_merge^T is then built with
    DVE/GPSIMD multiply-adds and fed straight into the head matmul as
    the stationary operand.
  - head matmul in bf16 with fp32 PSUM; PSUM evacuated by ACT copies to
    bf16 and DMA'd to DRAM bf16 (host converts to fp32).
  - software pipeline over token groups with a warm-up ramp
    (128/256/512... tokens): group g's head matmuls overlap group g+1's
    coeff pipeline and group g+2's logits, keeping the PE stream dense.
"""

import numpy as np

HC, C, TMAX = 4, 256, 8
RMS_EPS, PRE_EPS, SINK_EPS, POST_MULT = 1e-6, 1e-4, 1e-6, 2.0
VOCAB = 50257
B, S = 2, 2048
K = HC * C            # 1024
M = HC * HC + 2 * HC  # 24
NKC = K // 128        # 8 k-chunks
NCORES = 8
NT = B * S            # 4096
VS = 6283             # vocab rows per core (8*6283 = 50264 >= 50257)
VW = 512
NV = (VS + VW - 1) // VW          # 13 head tiles (12x512 + 139)
SCS = [128, 256] + [512] * 7 + [128]   # token-group ramp, sum = NT
OFF = [sum(SCS[:i]) for i in range(len(SCS))]
NG = len(SCS)
assert sum(SCS) == NT


def _build():
    from contextlib import ExitStack
    from concourse import bass, bacc, mybir
    import concourse.tile as tile
    from concourse.masks import make_identity

    f32 = mybir.dt.float32
    bf16 = mybir.dt.bfloat16
    AX = mybir.AxisListType
    OP = mybir.AluOpType
    AF = mybir.ActivationFunctionType

    nc = bacc.Bacc(target_bir_lowering=False)
    xt_p = nc.declare_dram_parameter("xt", [K, NT], bf16, False)
    wvt_p = nc.declare_dram_parameter("wvt", [K, VS], bf16, False)
    wit_p = nc.declare_dram_parameter("wit", [C, C], bf16, False)
    phi_p = nc.declare_dram_parameter("phi", [K, M], bf16, False)
    b_p = nc.declare_dram_parameter("b", [1, M], f32, False)
    al_p = nc.declare_dram_parameter("al", [1, 3], f32, False)
    out_p = nc.declare_dram_parameter("out", [NT, VS], bf16, True)

    with ExitStack() as ctx:
        tc = ctx.enter_context(tile.TileContext(nc))
        const = ctx.enter_context(tc.tile_pool(name="const", bufs=1))
        wtp = ctx.enter_context(tc.tile_pool(name="wtp", bufs=1))
        xtp = ctx.enter_context(tc.tile_pool(name="xtp", bufs=3))
        lgp = ctx.enter_context(tc.tile_pool(name="lgp", bufs=2))
        cfp = ctx.enter_context(tc.tile_pool(name="cfp", bufs=2))
        plp = ctx.enter_context(tc.tile_pool(name="plp", bufs=1))
        mxp = ctx.enter_context(tc.tile_pool(name="mxp", bufs=2))
        wkp = ctx.enter_context(tc.tile_pool(name="wkp", bufs=4))
        x2p = ctx.enter_context(tc.tile_pool(name="x2p", bufs=1))
        stp = ctx.enter_context(tc.tile_pool(name="stp", bufs=5))
        psh = ctx.enter_context(tc.tile_pool(name="psh", bufs=4, space="PSUM"))
        psa = ctx.enter_context(tc.tile_pool(name="psa", bufs=1, space="PSUM"))
        psb = ctx.enter_context(tc.tile_pool(name="psb", bufs=1, space="PSUM"))
        pst = ctx.enter_context(tc.tile_pool(name="pst", bufs=1, space="PSUM"))
        psf = ctx.enter_context(tc.tile_pool(name="psf", bufs=1, space="PSUM"))
        drp = ctx.enter_context(tc.tile_pool(name="drp", bufs=2, space="DRAM"))

        # ---------------- input prefetch (ahead of small const DMAs) ----
        # The first groups' xT tiles head the critical chain; issue their
        # DMAs before the descriptor-heavy little constant loads.
        xt_tiles = {}

        def prefetch_xt(g):
            gt, t0 = SCS[g], OFF[g]
            xtg = xtp.tile([128, NKC * gt], bf16, tag="xtg", name=f"xtg{g}")
            for kc in range(NKC):
                nc.sync.dma_start(
                    out=xtg[:, kc * gt:(kc + 1) * gt],
                    in_=xt_p[kc * 128:(kc + 1) * 128, t0:t0 + gt])
            xt_tiles[g] = xtg

        prefetch_xt(0)
        prefetch_xt(1)

        # ---------------- constants ----------------
        ident = const.tile([128, 128], bf16)
        make_identity(nc, ident[:])
        identf = const.tile([128, 128], f32)
        make_identity(nc, identf[:])

        cst = const.tile([128, 2], f32)
        nc.vector.memset(cst[:, 0:1], 0.0)
        nc.vector.memset(cst[:, 1:2], RMS_EPS)
        zero_b = cst[:, 0:1]
        eps_b = cst[:, 1:2]

        ones = const.tile([128, 1], bf16)
        nc.vector.memset(ones[:], 1.0)

        phi_sb = const.tile([128, NKC * M], bf16)
        for kc in range(NKC):
            nc.sync.dma_start(out=phi_sb[:, kc * M:(kc + 1) * M],
                              in_=phi_p[kc * 128:(kc + 1) * 128, :])
        b_bc = const.tile([128, M], f32)
        nc.sync.dma_start(out=b_bc[:], in_=b_p[0:1, :].to_broadcast([128, M]))
        al_bc = const.tile([128, 3], f32)
        nc.sync.dma_start(out=al_bc[:], in_=al_p[0:1, :].to_broadcast([128, 3]))

        # w_inner^T (k-major [c, o]) as 2 row bands
        wit_sb = const.tile([128, 2 * C], bf16)
        for h in range(2):
            nc.sync.dma_start(out=wit_sb[:, h * C:(h + 1) * C],
                              in_=wit_p[h * 128:(h + 1) * 128, :])

        # w_head^T slice: loaded in vocab-major stripes of 2 head tiles so
        # early head matmuls can start while later stripes stream in, and
        # split across both HWDGE queues for bandwidth.
        wt_all = wtp.tile([128, NKC * VS], bf16, tag="wt_all")
        NSTR = (VS + 2 * VW - 1) // (2 * VW)
        for sidx in range(NSTR):
            c0 = sidx * 2 * VW
            cw = min(2 * VW, VS - c0)
            eng = nc.scalar if sidx % 2 == 0 else nc.sync
            for kc in range(NKC):
                eng.dma_start(
                    out=wt_all[:, kc * VS + c0:kc * VS + c0 + cw],
                    in_=wvt_p[kc * 128:(kc + 1) * 128, c0:c0 + cw])

        st = {}  # per-group live tiles

        # ---------------- pipeline stages ----------------
        def stage_lg(g):
            """xT DMA, phi logits^T + sumsq row, transpose to token-major,
            RMS scale + coeff activations + Sinkhorn -> coefs."""
            gt, t0 = SCS[g], OFF[g]
            nch = gt // 128
            if g in xt_tiles:
                xtg = xt_tiles.pop(g)
            else:
                xtg = xtp.tile([128, NKC * gt], bf16, tag="xtg",
                               name=f"xtg{g}")
                for kc in range(NKC):
                    nc.sync.dma_start(
                        out=xtg[:, kc * gt:(kc + 1) * gt],
                        in_=xt_p[kc * 128:(kc + 1) * 128, t0:t0 + gt])

            # squares for the RMS sum (bf16 is plenty for the mean)
            x2s = []
            for half in range(2):
                x2 = x2p.tile([128, 4 * gt], bf16, tag=f"x2{half}",
                              name=f"x2_{g}_{half}")
                sl = slice(half * 4 * gt, (half + 1) * 4 * gt)
                nc.gpsimd.tensor_tensor(
                    out=x2[:], in0=xtg[:, sl], in1=xtg[:, sl], op=OP.mult)
                x2s.append(x2)

            pslg = psa.tile([32, gt], f32, tag="pslg")
            psss = psb.tile([32, gt], f32, tag="psss")
            for kc in range(NKC):
                nc.tensor.matmul(
                    out=pslg[0:M, :],
                    lhsT=phi_sb[:, kc * M:(kc + 1) * M],
                    rhs=xtg[:, kc * gt:(kc + 1) * gt],
                    start=(kc == 0), stop=(kc == NKC - 1))
            for kc in range(NKC):
                nc.tensor.matmul(
                    out=psss[0:1, :],
                    lhsT=ones[:],
                    rhs=x2s[kc // 4][:, (kc % 4) * gt:(kc % 4 + 1) * gt],
                    start=(kc == 0), stop=(kc == NKC - 1))

            lgsb = lgp.tile([32, gt], f32, tag="lgsb", name=f"lgsb{g}")
            nc.scalar.copy(lgsb[0:M, :], pslg[0:M, :])
            ssq = lgp.tile([1, gt], f32, tag="ssq", name=f"ssq{g}")
            nc.scalar.copy(ssq[0:1, :], psss[0:1, :])

            # token-major [128, nch, 24] + per-token sumsq column
            lgtm = lgp.tile([128, nch * 32], f32, tag="lgtm", name=f"lgtm{g}")
            msq = lgp.tile([128, nch], f32, tag="msq", name=f"msq{g}")
            for tcx in range(nch):
                pT = pst.tile([128, 128], f32, tag="psT")
                nc.tensor.transpose(
                    out=pT[:, 0:M],
                    in_=lgsb[0:M, tcx * 128:(tcx + 1) * 128],
                    identity=identf[0:M, 0:M])
                nc.tensor.transpose(
                    out=pT[:, M:M + 1],
                    in_=ssq[0:1, tcx * 128:(tcx + 1) * 128],
                    identity=identf[0:1, 0:1])
                nc.scalar.copy(lgtm[:, tcx * 32:tcx * 32 + M], pT[:, 0:M])
                nc.scalar.copy(msq[:, tcx:tcx + 1], pT[:, M:M + 1])
            lgv = lgtm[:].rearrange("p (c w) -> p c w", w=32)

            # scl = 1/sqrt(mean+eps)
            scl = lgp.tile([128, nch], f32, tag="scl", name=f"scl{g}")
            nc.scalar.activation(out=scl[:], in_=msq[:],
                                 func=AF.Sqrt, scale=1.0 / K, bias=eps_b)
            nc.vector.reciprocal(scl[:], scl[:])
            for tcx in range(nch):
                nc.vector.tensor_scalar_mul(
                    lgv[:, tcx, 0:M], lgv[:, tcx, 0:M], scl[:, tcx:tcx + 1])
            nc.vector.tensor_tensor(
                out=lgv[:, :, 0:M], in0=lgv[:, :, 0:M],
                in1=b_bc[:][:, None, :].to_broadcast([128, nch, M]), op=OP.add)

            # coefs [128, nch, 24]: [0:16]=exp(res), [16:20]=h_pre,
            # [20:24]=h_post2
            coefs = cfp.tile([128, nch * M], f32, tag="coefs",
                             name=f"coefs{g}")
            cfv = coefs[:].rearrange("p (c m) -> p c m", m=M)
            nc.scalar.activation(out=cfv[:, :, 16:20], in_=lgv[:, :, 0:4],
                                 func=AF.Sigmoid, bias=zero_b,
                                 scale=al_bc[:, 0:1])
            nc.vector.tensor_scalar_add(cfv[:, :, 16:20], cfv[:, :, 16:20],
                                        PRE_EPS)
            nc.scalar.activation(out=cfv[:, :, 20:24], in_=lgv[:, :, 4:8],
                                 func=AF.Sigmoid, bias=zero_b,
                                 scale=al_bc[:, 1:2])
            nc.vector.tensor_scalar_mul(cfv[:, :, 20:24], cfv[:, :, 20:24],
                                        POST_MULT)
            nc.scalar.activation(out=cfv[:, :, 0:16], in_=lgv[:, :, 8:24],
                                 func=AF.Exp, bias=zero_b, scale=al_bc[:, 2:3])

            # batched Sinkhorn on cfv[:, :, 0:16].
            # SINK_EPS (1e-6 vs O(1) row sums) is dropped: it shifts the
            # result by ~1e-6 relative, far below the bf16 noise floor.
            mv4 = cfv[:, :, 0:16].rearrange("p c (o i) -> p c o i", i=4)
            mv4t = cfv[:, :, 0:16].rearrange("p c (o i) -> p c i o", i=4)
            for _ in range(TMAX):
                rs = wkp.tile([128, 4 * 4], f32, tag="rs")
                rsv = rs[:, 0:nch * 4].rearrange("p (c o) -> p c o", c=nch)
                nc.vector.tensor_reduce(rsv, mv4, axis=AX.X, op=OP.add)
                nc.vector.reciprocal(rs[:, 0:nch * 4], rs[:, 0:nch * 4])
                nc.vector.tensor_tensor(
                    out=mv4, in0=mv4,
                    in1=rsv[:, :, :, None].to_broadcast([128, nch, 4, 4]),
                    op=OP.mult)
                cs = wkp.tile([128, 4 * 4], f32, tag="cs")
                csv = cs[:, 0:nch * 4].rearrange("p (c i) -> p c i", c=nch)
                nc.vector.tensor_reduce(csv, mv4t, axis=AX.X, op=OP.add)
                nc.vector.reciprocal(cs[:, 0:nch * 4], cs[:, 0:nch * 4])
                nc.vector.tensor_tensor(
                    out=mv4, in0=mv4,
                    in1=csv[:, :, None, :].to_broadcast([128, nch, 4, 4]),
                    op=OP.mult)
            st[g] = dict(xtg=xtg, coefs=coefs)

        def stage_planes(g):
            """Transpose coefs back to [24, T]; bounce through DRAM and
            broadcast-read -> planes [128, 24*gt]."""
            gt = SCS[g]
            nch = gt // 128
            coefs = st[g]["coefs"]
            ctstg = cfp.tile([32, gt], bf16, tag="ctstg", name=f"ctstg{g}")
            for tcx in range(nch):
                pT = pst.tile([128, 128], f32, tag="psT")
                nc.tensor.transpose(
                    out=pT[0:M, 0:128],
                    in_=coefs[:, tcx * M:(tcx + 1) * M],
                    identity=identf[:, 0:128])
                nc.scalar.copy(
                    ctstg[0:M, tcx * 128:(tcx + 1) * 128], pT[0:M, 0:128])
            dtile = drp.tile([1, M * gt], bf16, tag="cfdram",
                             name=f"cfdram{g}")
            nc.sync.dma_start(
                out=dtile[0:1, :].rearrange("x (c t) -> (x c) t", c=M),
                in_=ctstg[0:M, :])
            planes = plp.tile([128, M * gt], bf16, tag="planes",
                              name=f"planes{g}")
            nc.sync.dma_start(
                out=planes[:],
                in_=dtile[0:1, :].to_broadcast([128, M * gt]))
            st[g]["planes"] = planes
            # x_in^T = sum_i h_pre[i] * x^T[i]  (2 half-chunks of c)
            xtg = st[g]["xtg"]
            xin = mxp.tile([128, 2 * gt], bf16, tag="xin", name=f"xin{g}")
            for h in range(2):
                seg = xin[:, h * gt:(h + 1) * gt]
                nc.vector.tensor_tensor(
                    out=seg, in0=xtg[:, h * gt:(h + 1) * gt],
                    in1=planes[:, 16 * gt:17 * gt], op=OP.mult)
                for i in range(1, HC):
                    tmp = wkp.tile([128, 512], bf16, tag="tmp")
                    nc.vector.tensor_tensor(
                        out=tmp[:, 0:gt],
                        in0=xtg[:, (i * 2 + h) * gt:(i * 2 + h + 1) * gt],
                        in1=planes[:, (16 + i) * gt:(17 + i) * gt],
                        op=OP.mult)
                    eng = nc.vector if i % 2 else nc.gpsimd
                    eng.tensor_add(seg, seg, tmp[:, 0:gt])
            st[g]["xin"] = xin

        def stage_fo(g):
            """f_out^T = w_inner @ x_in^T : 2 o-blocks x 2 c-halves."""
            gt = SCS[g]
            xin = st[g]["xin"]
            fo = mxp.tile([128, 2 * gt], bf16, tag="fo", name=f"fo{g}")
            for ob in range(2):
                pf = psf.tile([128, gt], f32, tag="psf")
                for h in range(2):
                    nc.tensor.matmul(
                        out=pf[:],
                        lhsT=wit_sb[:, h * C + ob * 128:h * C + (ob + 1) * 128],
                        rhs=xin[:, h * gt:(h + 1) * gt],
                        start=(h == 0), stop=(h == 1))
                nc.scalar.copy(fo[:, ob * gt:(ob + 1) * gt], pf[:])
            st[g]["fo"] = fo

        def stage_mix(g):
            """x_merge^T[kc] = sum_i res[o,i]*x^T[i,h] + post2[o]*f_out^T[h]"""
            gt = SCS[g]
            xtg, planes, fo = st[g]["xtg"], st[g]["planes"], st[g]["fo"]
            xmg = mxp.tile([128, NKC * gt], bf16, tag="xmg", name=f"xmg{g}")
            for kc in range(NKC):
                o, h = kc // 2, kc % 2
                seg = xmg[:, kc * gt:(kc + 1) * gt]
                nc.vector.tensor_tensor(
                    out=seg, in0=xtg[:, h * gt:(h + 1) * gt],
                    in1=planes[:, (o * 4) * gt:(o * 4 + 1) * gt], op=OP.mult)
                for i in range(1, HC):
                    tmp = wkp.tile([128, 512], bf16, tag="tmp")
                    nc.vector.tensor_tensor(
                        out=tmp[:, 0:gt],
                        in0=xtg[:, (i * 2 + h) * gt:(i * 2 + h + 1) * gt],
                        in1=planes[:, (o * 4 + i) * gt:(o * 4 + i + 1) * gt],
                        op=OP.mult)
                    eng = nc.vector if i % 2 else nc.gpsimd
                    eng.tensor_add(seg, seg, tmp[:, 0:gt])
                tmp = wkp.tile([128, 512], bf16, tag="tmp")
                nc.vector.tensor_tensor(
                    out=tmp[:, 0:gt], in0=fo[:, h * gt:(h + 1) * gt],
                    in1=planes[:, (20 + o) * gt:(21 + o) * gt], op=OP.mult)
                nc.gpsimd.tensor_add(seg, seg, tmp[:, 0:gt])
            st[g]["xmg"] = xmg

        def head_chunk(g, tcx):
            gt = SCS[g]
            xmg = st[g]["xmg"]
            t0 = OFF[g] + tcx * 128
            stg = None
            for v in range(NV):
                w = min(VW, VS - v * VW)
                ph = psh.tile([128, VW], f32, tag="psh")
                for kc in range(NKC):
                    nc.tensor.matmul(
                        out=ph[:, 0:w],
                        lhsT=xmg[:, kc * gt + tcx * 128:
                                 kc * gt + (tcx + 1) * 128],
                        rhs=wt_all[:, kc * VS + v * VW:kc * VS + v * VW + w],
                        start=(kc == 0), stop=(kc == NKC - 1))
                # pair two v-tiles per staging tile / output DMA
                half = v % 2
                if half == 0:
                    stg = stp.tile([128, 2 * VW], bf16, tag="stg")
                nc.scalar.copy(stg[:, half * VW:half * VW + w], ph[:, 0:w])
                if half == 1 or v == NV - 1:
                    v0 = v - half
                    ww = min(2 * VW, VS - v0 * VW)
                    nc.sync.dma_start(
                        out=out_p[t0:t0 + 128, v0 * VW:v0 * VW + ww],
                        in_=stg[:, 0:ww])

        # ---------------- emission (software pipeline) ----------------
        stage_lg(0)
        stage_lg(1)
        stage_planes(0)
        stage_fo(0)
        stage_mix(0)
        for g in range(NG):
            nch = SCS[g] // 128
            inserts = {0: [], 1: [], 2: []}
            if g + 1 < NG:
                inserts[0].append(("planes", g + 1))
                inserts[min(1, nch - 1)].append(("fo", g + 1))
                inserts[min(1, nch - 1)].append(("mix", g + 1))
            if g + 2 < NG:
                inserts[min(2, nch - 1)].append(("lg", g + 2))
            for tcx in range(nch):
                head_chunk(g, tcx)
                for kind, gg in inserts.get(tcx, []):
                    if kind == "planes":
                        stage_planes(gg)
                    elif kind == "fo":
                        stage_fo(gg)
                    elif kind == "mix":
                        stage_mix(gg)
                    else:
                        stage_lg(gg)
            del st[g]

    if not nc.is_finalized():
        nc.finalize()
    return nc


_NC_CACHE = {}


def _get_nc():
    if "nc" not in _NC_CACHE:
        _NC_CACHE["nc"] = _build()
    return _NC_CACHE["nc"]


def _make_in_maps(input_ids, embed, w_inner, w_head, phi, b,
                  alpha_pre, alpha_post, alpha_res):
    import ml_dtypes
    bf = ml_dtypes.bfloat16

    ids = np.asarray(input_ids).reshape(-1).astype(np.int64)
    x = np.asarray(embed)[ids].astype(bf)                 # [NT, K]
    xt = np.ascontiguousarray(x.T)                        # [K, NT]
    phi_np = np.ascontiguousarray(np.asarray(phi).astype(bf))
    wit = np.ascontiguousarray(np.asarray(w_inner).astype(bf).T)  # [c, o]
    b_np = np.ascontiguousarray(np.asarray(b, dtype=np.float32).reshape(1, M))
    al = np.array([[np.asarray(alpha_pre).reshape(-1)[0],
                    np.asarray(alpha_post).reshape(-1)[0],
                    np.asarray(alpha_res).reshape(-1)[0]]], dtype=np.float32)
    wh = np.asarray(w_head).astype(bf)                    # [VOCAB, K]

    in_maps = []
    for i in range(NCORES):
        sl = wh[i * VS:(i + 1) * VS]                      # [<=VS, K]
        wvt = np.zeros((K, VS), bf)
        wvt[:, :sl.shape[0]] = sl.T
        in_maps.append(dict(xt=xt, wvt=np.ascontiguousarray(wvt),
                            wit=wit, phi=phi_np, b=b_np, al=al))
    return in_maps


def _run(in_maps, trace=False):
    from concourse.bass_utils import run_bass_kernel_spmd
    nc = _get_nc()
    return run_bass_kernel_spmd(nc, in_maps, list(range(NCORES)), trace=trace)


def kernel(input_ids, embed, w_inner, w_head, phi, b,
           alpha_pre, alpha_post, alpha_res):
    in_maps = _make_in_maps(input_ids, embed, w_inner, w_head, phi, b,
                            alpha_pre, alpha_post, alpha_res)
    res = _run(in_maps).results
    out = np.concatenate([np.asarray(res[i]["out"]) for i in range(NCORES)],
                         axis=1)[:, :VOCAB]
    return np.ascontiguousarray(out.reshape(B, S, VOCAB).astype(np.float32))


# revision 35
# speedup vs baseline: 1.0421x; 1.0048x over previous
"""Trainium2 Bass kernel for nn_MiniMHCLM (moe_routing).

Strategy (8 NeuronCores, SPMD, no collectives):
  - vocab-sharded head matmul: core i holds w_head rows [i*VS:(i+1)*VS]
    (host-sliced, zero-padded to uniform VS) transposed to k-major bf16;
    it computes logits for all 4096 tokens x its vocab slice and the host
    concatenates along vocab.
  - token embeddings are pre-gathered AND pre-transposed on the host into
    xT [K, NT] bf16 (numerically identical to embed[ids].astype(bf16)),
    so the device pipeline runs fully k-major with zero PE transposes of x
    and no indirect DMA.
  - per-token coeffs: phi-stationary matmul gives logits^T [24, T] plus a
    ones-matmul row of sum(x^2); one small PE transpose per 128-token
    chunk moves both to token-major for the RMS scale, sigmoid/exp and
    the Sinkhorn iterations (DVE/ACT, batched per token group).
  - mixing runs transposed: per-token coeffs are PE-transposed back to
    [24, T], bounced through a DRAM scratch row and broadcast across
    partitions with a stride-0 DMA read; x_merge^T is then built with
    DVE/GPSIMD multiply-adds and fed straight into the head matmul as
    the stationary operand.
  - head matmul in bf16 with fp32 PSUM; PSUM evacuated by ACT copies to
    bf16 and DMA'd to DRAM bf16 (host converts to fp32).
  - software pipeline over token groups with a warm-up ramp
    (128/256/512... tokens): group g's head matmuls overlap group g+1's
    coeff pipeline and group g+2's logits, keeping the PE stream dense.
"""

import numpy as np

HC, C, TMAX = 4, 256, 8
RMS_EPS, PRE_EPS, SINK_EPS, POST_MULT = 1e-6, 1e-4, 1e-6, 2.0
VOCAB = 50257
B, S = 2, 2048
K = HC * C            # 1024
M = HC * HC + 2 * HC  # 24
NKC = K // 128        # 8 k-chunks
NCORES = 8
NT = B * S            # 4096
VS = 6283             # vocab rows per core (8*6283 = 50264 >= 50257)
VW = 512
NV = (VS + VW - 1) // VW          # 13 head tiles (12x512 + 139)
SCS = [128, 256] + [512] * 7 + [128]   # token-group ramp, sum = NT
OFF = [sum(SCS[:i]) for i in range(len(SCS))]
NG = len(SCS)
assert sum(SCS) == NT


def _build():
    from contextlib import ExitStack
    from concourse import bass, bacc, mybir
    import concourse.tile as tile
    from concourse.masks import make_identity

    f32 = mybir.dt.float32
    bf16 = mybir.dt.bfloat16
    AX = mybir.AxisListType
    OP = mybir.AluOpType
    AF = mybir.ActivationFunctionType

    nc = bacc.Bacc(target_bir_lowering=False)
    xt_p = nc.declare_dram_parameter("xt", [K, NT], bf16, False)
    wvt_p = nc.declare_dram_parameter("wvt", [K, VS], bf16, False)
    wit_p = nc.declare_dram_parameter("wit", [C, C], bf16, False)
    phi_p = nc.declare_dram_parameter("phi", [K, M], bf16, False)
    b_p = nc.declare_dram_parameter("b", [1, M], f32, False)
    al_p = nc.declare_dram_parameter("al", [1, 3], f32, False)
    out_p = nc.declare_dram_parameter("out", [NT, VS], bf16, True)

    with ExitStack() as ctx:
        tc = ctx.enter_context(tile.TileContext(nc))
        const = ctx.enter_context(tc.tile_pool(name="const", bufs=1))
        wtp = ctx.enter_context(tc.tile_pool(name="wtp", bufs=1))
        xtp = ctx.enter_context(tc.tile_pool(name="xtp", bufs=3))
        lgp = ctx.enter_context(tc.tile_pool(name="lgp", bufs=2))
        cfp = ctx.enter_context(tc.tile_pool(name="cfp", bufs=2))
        plp = ctx.enter_context(tc.tile_pool(name="plp", bufs=1))
        mxp = ctx.enter_context(tc.tile_pool(name="mxp", bufs=2))
        wkp = ctx.enter_context(tc.tile_pool(name="wkp", bufs=4))
        x2p = ctx.enter_context(tc.tile_pool(name="x2p", bufs=1))
        stp = ctx.enter_context(tc.tile_pool(name="stp", bufs=5))
        psh = ctx.enter_context(tc.tile_pool(name="psh", bufs=4, space="PSUM"))
        psa = ctx.enter_context(tc.tile_pool(name="psa", bufs=1, space="PSUM"))
        psb = ctx.enter_context(tc.tile_pool(name="psb", bufs=1, space="PSUM"))
        pst = ctx.enter_context(tc.tile_pool(name="pst", bufs=1, space="PSUM"))
        psf = ctx.enter_context(tc.tile_pool(name="psf", bufs=1, space="PSUM"))
        drp = ctx.enter_context(tc.tile_pool(name="drp", bufs=2, space="DRAM"))

        # ---------------- input prefetch (ahead of small const DMAs) ----
        # The first groups' xT tiles head the critical chain; issue their
        # DMAs before the descriptor-heavy little constant loads.
        xt_tiles = {}

        def prefetch_xt(g):
            gt, t0 = SCS[g], OFF[g]
            xtg = xtp.tile([128, NKC * gt], bf16, tag="xtg", name=f"xtg{g}")
            for kc in range(NKC):
                nc.sync.dma_start(
                    out=xtg[:, kc * gt:(kc + 1) * gt],
                    in_=xt_p[kc * 128:(kc + 1) * 128, t0:t0 + gt])
            xt_tiles[g] = xtg

        prefetch_xt(0)
        prefetch_xt(1)

        # ---------------- constants ----------------
        ident = const.tile([128, 128], bf16)
        make_identity(nc, ident[:])
        identf = const.tile([128, 128], f32)
        make_identity(nc, identf[:])

        cst = const.tile([128, 2], f32)
        nc.vector.memset(cst[:, 0:1], 0.0)
        nc.vector.memset(cst[:, 1:2], RMS_EPS)
        zero_b = cst[:, 0:1]
        eps_b = cst[:, 1:2]

        ones = const.tile([128, 1], bf16)
        nc.vector.memset(ones[:], 1.0)

        phi_sb = const.tile([128, NKC * M], bf16)
        for kc in range(NKC):
            nc.sync.dma_start(out=phi_sb[:, kc * M:(kc + 1) * M],
                              in_=phi_p[kc * 128:(kc + 1) * 128, :])
        b_bc = const.tile([128, M], f32)
        nc.sync.dma_start(out=b_bc[:], in_=b_p[0:1, :].to_broadcast([128, M]))
        al_bc = const.tile([128, 3], f32)
        nc.sync.dma_start(out=al_bc[:], in_=al_p[0:1, :].to_broadcast([128, 3]))

        # w_inner^T (k-major [c, o]) as 2 row bands
        wit_sb = const.tile([128, 2 * C], bf16)
        for h in range(2):
            nc.sync.dma_start(out=wit_sb[:, h * C:(h + 1) * C],
                              in_=wit_p[h * 128:(h + 1) * 128, :])

        # w_head^T slice: loaded in vocab-major stripes of 2 head tiles so
        # early head matmuls can start while later stripes stream in, and
        # split across both HWDGE queues for bandwidth.
        wt_all = wtp.tile([128, NKC * VS], bf16, tag="wt_all")
        NSTR = (VS + 2 * VW - 1) // (2 * VW)
        for sidx in range(NSTR):
            c0 = sidx * 2 * VW
            cw = min(2 * VW, VS - c0)
            eng = nc.scalar if sidx % 2 == 0 else nc.sync
            for kc in range(NKC):
                eng.dma_start(
                    out=wt_all[:, kc * VS + c0:kc * VS + c0 + cw],
                    in_=wvt_p[kc * 128:(kc + 1) * 128, c0:c0 + cw])

        st = {}  # per-group live tiles

        # ---------------- pipeline stages ----------------
        def stage_lg(g):
            """xT DMA, phi logits^T + sumsq row, transpose to token-major,
            RMS scale + coeff activations + Sinkhorn -> coefs."""
            gt, t0 = SCS[g], OFF[g]
            nch = gt // 128
            if g in xt_tiles:
                xtg = xt_tiles.pop(g)
            else:
                xtg = xtp.tile([128, NKC * gt], bf16, tag="xtg",
                               name=f"xtg{g}")
                for kc in range(NKC):
                    nc.sync.dma_start(
                        out=xtg[:, kc * gt:(kc + 1) * gt],
                        in_=xt_p[kc * 128:(kc + 1) * 128, t0:t0 + gt])

            # squares for the RMS sum (bf16 is plenty for the mean)
            x2s = []
            for half in range(2):
                x2 = x2p.tile([128, 4 * gt], bf16, tag=f"x2{half}",
                              name=f"x2_{g}_{half}")
                sl = slice(half * 4 * gt, (half + 1) * 4 * gt)
                nc.vector.tensor_tensor(
                    out=x2[:], in0=xtg[:, sl], in1=xtg[:, sl], op=OP.mult)
                x2s.append(x2)

            pslg = psa.tile([32, gt], f32, tag="pslg")
            psss = psb.tile([32, gt], f32, tag="psss")
            for kc in range(NKC):
                nc.tensor.matmul(
                    out=pslg[0:M, :],
                    lhsT=phi_sb[:, kc * M:(kc + 1) * M],
                    rhs=xtg[:, kc * gt:(kc + 1) * gt],
                    start=(kc == 0), stop=(kc == NKC - 1))
            for kc in range(NKC):
                nc.tensor.matmul(
                    out=psss[0:1, :],
                    lhsT=ones[:],
                    rhs=x2s[kc // 4][:, (kc % 4) * gt:(kc % 4 + 1) * gt],
                    start=(kc == 0), stop=(kc == NKC - 1))

            lgsb = lgp.tile([32, gt], f32, tag="lgsb", name=f"lgsb{g}")
            nc.scalar.copy(lgsb[0:M, :], pslg[0:M, :])
            ssq = lgp.tile([1, gt], f32, tag="ssq", name=f"ssq{g}")
            nc.scalar.copy(ssq[0:1, :], psss[0:1, :])

            # token-major [128, nch, 24] + per-token sumsq column
            lgtm = lgp.tile([128, nch * 32], f32, tag="lgtm", name=f"lgtm{g}")
            msq = lgp.tile([128, nch], f32, tag="msq", name=f"msq{g}")
            for tcx in range(nch):
                pT = pst.tile([128, 128], f32, tag="psT")
                nc.tensor.transpose(
                    out=pT[:, 0:M],
                    in_=lgsb[0:M, tcx * 128:(tcx + 1) * 128],
                    identity=identf[0:M, 0:M])
                nc.tensor.transpose(
                    out=pT[:, M:M + 1],
                    in_=ssq[0:1, tcx * 128:(tcx + 1) * 128],
                    identity=identf[0:1, 0:1])
                nc.scalar.copy(lgtm[:, tcx * 32:tcx * 32 + M], pT[:, 0:M])
                nc.scalar.copy(msq[:, tcx:tcx + 1], pT[:, M:M + 1])
            lgv = lgtm[:].rearrange("p (c w) -> p c w", w=32)

            # scl = 1/sqrt(mean+eps)
            scl = lgp.tile([128, nch], f32, tag="scl", name=f"scl{g}")
            nc.scalar.activation(out=scl[:], in_=msq[:],
                                 func=AF.Sqrt, scale=1.0 / K, bias=eps_b)
            nc.vector.reciprocal(scl[:], scl[:])
            for tcx in range(nch):
                nc.vector.tensor_scalar_mul(
                    lgv[:, tcx, 0:M], lgv[:, tcx, 0:M], scl[:, tcx:tcx + 1])
            nc.vector.tensor_tensor(
                out=lgv[:, :, 0:M], in0=lgv[:, :, 0:M],
                in1=b_bc[:][:, None, :].to_broadcast([128, nch, M]), op=OP.add)

            # coefs [128, nch, 24]: [0:16]=exp(res), [16:20]=h_pre,
            # [20:24]=h_post2
            coefs = cfp.tile([128, nch * M], f32, tag="coefs",
                             name=f"coefs{g}")
            cfv = coefs[:].rearrange("p (c m) -> p c m", m=M)
            nc.scalar.activation(out=cfv[:, :, 16:20], in_=lgv[:, :, 0:4],
                                 func=AF.Sigmoid, bias=zero_b,
                                 scale=al_bc[:, 0:1])
            nc.vector.tensor_scalar_add(cfv[:, :, 16:20], cfv[:, :, 16:20],
                                        PRE_EPS)
            nc.scalar.activation(out=cfv[:, :, 20:24], in_=lgv[:, :, 4:8],
                                 func=AF.Sigmoid, bias=zero_b,
                                 scale=al_bc[:, 1:2])
            nc.vector.tensor_scalar_mul(cfv[:, :, 20:24], cfv[:, :, 20:24],
                                        POST_MULT)
            nc.scalar.activation(out=cfv[:, :, 0:16], in_=lgv[:, :, 8:24],
                                 func=AF.Exp, bias=zero_b, scale=al_bc[:, 2:3])

            # batched Sinkhorn on cfv[:, :, 0:16].
            # SINK_EPS (1e-6 vs O(1) row sums) is dropped: it shifts the
            # result by ~1e-6 relative, far below the bf16 noise floor.
            mv4 = cfv[:, :, 0:16].rearrange("p c (o i) -> p c o i", i=4)
            mv4t = cfv[:, :, 0:16].rearrange("p c (o i) -> p c i o", i=4)
            for _ in range(TMAX):
                rs = wkp.tile([128, 4 * 4], f32, tag="rs")
                rsv = rs[:, 0:nch * 4].rearrange("p (c o) -> p c o", c=nch)
                nc.vector.tensor_reduce(rsv, mv4, axis=AX.X, op=OP.add)
                nc.vector.reciprocal(rs[:, 0:nch * 4], rs[:, 0:nch * 4])
                nc.vector.tensor_tensor(
                    out=mv4, in0=mv4,
                    in1=rsv[:, :, :, None].to_broadcast([128, nch, 4, 4]),
                    op=OP.mult)
                cs = wkp.tile([128, 4 * 4], f32, tag="cs")
                csv = cs[:, 0:nch * 4].rearrange("p (c i) -> p c i", c=nch)
                nc.vector.tensor_reduce(csv, mv4t, axis=AX.X, op=OP.add)
                nc.vector.reciprocal(cs[:, 0:nch * 4], cs[:, 0:nch * 4])
                nc.vector.tensor_tensor(
                    out=mv4, in0=mv4,
                    in1=csv[:, :, None, :].to_broadcast([128, nch, 4, 4]),
                    op=OP.mult)
            st[g] = dict(xtg=xtg, coefs=coefs)

        def stage_planes(g):
            """Transpose coefs back to [24, T]; bounce through DRAM and
            broadcast-read -> planes [128, 24*gt]."""
            gt = SCS[g]
            nch = gt // 128
            coefs = st[g]["coefs"]
            ctstg = cfp.tile([32, gt], bf16, tag="ctstg", name=f"ctstg{g}")
            for tcx in range(nch):
                pT = pst.tile([128, 128], f32, tag="psT")
                nc.tensor.transpose(
                    out=pT[0:M, 0:128],
                    in_=coefs[:, tcx * M:(tcx + 1) * M],
                    identity=identf[:, 0:128])
                nc.scalar.copy(
                    ctstg[0:M, tcx * 128:(tcx + 1) * 128], pT[0:M, 0:128])
            dtile = drp.tile([1, M * gt], bf16, tag="cfdram",
                             name=f"cfdram{g}")
            nc.sync.dma_start(
                out=dtile[0:1, :].rearrange("x (c t) -> (x c) t", c=M),
                in_=ctstg[0:M, :])
            planes = plp.tile([128, M * gt], bf16, tag="planes",
                              name=f"planes{g}")
            nc.sync.dma_start(
                out=planes[:],
                in_=dtile[0:1, :].to_broadcast([128, M * gt]))
            st[g]["planes"] = planes
            # x_in^T = sum_i h_pre[i] * x^T[i]  (2 half-chunks of c)
            xtg = st[g]["xtg"]
            xin = mxp.tile([128, 2 * gt], bf16, tag="xin", name=f"xin{g}")
            for h in range(2):
                seg = xin[:, h * gt:(h + 1) * gt]
                nc.vector.tensor_tensor(
                    out=seg, in0=xtg[:, h * gt:(h + 1) * gt],
                    in1=planes[:, 16 * gt:17 * gt], op=OP.mult)
                for i in range(1, HC):
                    tmp = wkp.tile([128, 512], bf16, tag="tmp")
                    nc.vector.tensor_tensor(
                        out=tmp[:, 0:gt],
                        in0=xtg[:, (i * 2 + h) * gt:(i * 2 + h + 1) * gt],
                        in1=planes[:, (16 + i) * gt:(17 + i) * gt],
                        op=OP.mult)
                    eng = nc.vector if i % 2 else nc.gpsimd
                    eng.tensor_add(seg, seg, tmp[:, 0:gt])
            st[g]["xin"] = xin

        def stage_fo(g):
            """f_out^T = w_inner @ x_in^T : 2 o-blocks x 2 c-halves."""
            gt = SCS[g]
            xin = st[g]["xin"]
            fo = mxp.tile([128, 2 * gt], bf16, tag="fo", name=f"fo{g}")
            for ob in range(2):
                pf = psf.tile([128, gt], f32, tag="psf")
                for h in range(2):
                    nc.tensor.matmul(
                        out=pf[:],
                        lhsT=wit_sb[:, h * C + ob * 128:h * C + (ob + 1) * 128],
                        rhs=xin[:, h * gt:(h + 1) * gt],
                        start=(h == 0), stop=(h == 1))
                nc.scalar.copy(fo[:, ob * gt:(ob + 1) * gt], pf[:])
            st[g]["fo"] = fo

        def stage_mix(g):
            """x_merge^T[kc] = sum_i res[o,i]*x^T[i,h] + post2[o]*f_out^T[h]"""
            gt = SCS[g]
            xtg, planes, fo = st[g]["xtg"], st[g]["planes"], st[g]["fo"]
            xmg = mxp.tile([128, NKC * gt], bf16, tag="xmg", name=f"xmg{g}")
            for kc in range(NKC):
                o, h = kc // 2, kc % 2
                seg = xmg[:, kc * gt:(kc + 1) * gt]
                nc.vector.tensor_tensor(
                    out=seg, in0=xtg[:, h * gt:(h + 1) * gt],
                    in1=planes[:, (o * 4) * gt:(o * 4 + 1) * gt], op=OP.mult)
                for i in range(1, HC):
                    tmp = wkp.tile([128, 512], bf16, tag="tmp")
                    nc.vector.tensor_tensor(
                        out=tmp[:, 0:gt],
                        in0=xtg[:, (i * 2 + h) * gt:(i * 2 + h + 1) * gt],
                        in1=planes[:, (o * 4 + i) * gt:(o * 4 + i + 1) * gt],
                        op=OP.mult)
                    eng = nc.vector if i % 2 else nc.gpsimd
                    eng.tensor_add(seg, seg, tmp[:, 0:gt])
                tmp = wkp.tile([128, 512], bf16, tag="tmp")
                nc.vector.tensor_tensor(
                    out=tmp[:, 0:gt], in0=fo[:, h * gt:(h + 1) * gt],
                    in1=planes[:, (20 + o) * gt:(21 + o) * gt], op=OP.mult)
                nc.gpsimd.tensor_add(seg, seg, tmp[:, 0:gt])
            st[g]["xmg"] = xmg

        def head_chunk(g, tcx):
            gt = SCS[g]
            xmg = st[g]["xmg"]
            t0 = OFF[g] + tcx * 128
            stg = None
            for v in range(NV):
                w = min(VW, VS - v * VW)
                ph = psh.tile([128, VW], f32, tag="psh")
                for kc in range(NKC):
                    nc.tensor.matmul(
                        out=ph[:, 0:w],
                        lhsT=xmg[:, kc * gt + tcx * 128:
                                 kc * gt + (tcx + 1) * 128],
                        rhs=wt_all[:, kc * VS + v * VW:kc * VS + v * VW + w],
                        start=(kc == 0), stop=(kc == NKC - 1))
                # pair two v-tiles per staging tile / output DMA
                half = v % 2
                if half == 0:
                    stg = stp.tile([128, 2 * VW], bf16, tag="stg")
                nc.scalar.copy(stg[:, half * VW:half * VW + w], ph[:, 0:w])
                if half == 1 or v == NV - 1:
                    v0 = v - half
                    ww = min(2 * VW, VS - v0 * VW)
                    nc.sync.dma_start(
                        out=out_p[t0:t0 + 128, v0 * VW:v0 * VW + ww],
                        in_=stg[:, 0:ww])

        # ---------------- emission (software pipeline) ----------------
        stage_lg(0)
        stage_lg(1)
        stage_planes(0)
        stage_fo(0)
        stage_mix(0)
        for g in range(NG):
            nch = SCS[g] // 128
            inserts = {0: [], 1: [], 2: []}
            if g + 1 < NG:
                inserts[0].append(("planes", g + 1))
                inserts[min(1, nch - 1)].append(("fo", g + 1))
                inserts[min(1, nch - 1)].append(("mix", g + 1))
            if g + 2 < NG:
                inserts[min(2, nch - 1)].append(("lg", g + 2))
            for tcx in range(nch):
                head_chunk(g, tcx)
                for kind, gg in inserts.get(tcx, []):
                    if kind == "planes":
                        stage_planes(gg)
                    elif kind == "fo":
                        stage_fo(gg)
                    elif kind == "mix":
                        stage_mix(gg)
                    else:
                        stage_lg(gg)
            del st[g]

    if not nc.is_finalized():
        nc.finalize()
    return nc


_NC_CACHE = {}


def _get_nc():
    if "nc" not in _NC_CACHE:
        _NC_CACHE["nc"] = _build()
    return _NC_CACHE["nc"]


def _make_in_maps(input_ids, embed, w_inner, w_head, phi, b,
                  alpha_pre, alpha_post, alpha_res):
    import ml_dtypes
    bf = ml_dtypes.bfloat16

    ids = np.asarray(input_ids).reshape(-1).astype(np.int64)
    x = np.asarray(embed)[ids].astype(bf)                 # [NT, K]
    xt = np.ascontiguousarray(x.T)                        # [K, NT]
    phi_np = np.ascontiguousarray(np.asarray(phi).astype(bf))
    wit = np.ascontiguousarray(np.asarray(w_inner).astype(bf).T)  # [c, o]
    b_np = np.ascontiguousarray(np.asarray(b, dtype=np.float32).reshape(1, M))
    al = np.array([[np.asarray(alpha_pre).reshape(-1)[0],
                    np.asarray(alpha_post).reshape(-1)[0],
                    np.asarray(alpha_res).reshape(-1)[0]]], dtype=np.float32)
    wh = np.asarray(w_head).astype(bf)                    # [VOCAB, K]

    in_maps = []
    for i in range(NCORES):
        sl = wh[i * VS:(i + 1) * VS]                      # [<=VS, K]
        wvt = np.zeros((K, VS), bf)
        wvt[:, :sl.shape[0]] = sl.T
        in_maps.append(dict(xt=xt, wvt=np.ascontiguousarray(wvt),
                            wit=wit, phi=phi_np, b=b_np, al=al))
    return in_maps


def _run(in_maps, trace=False):
    from concourse.bass_utils import run_bass_kernel_spmd
    nc = _get_nc()
    return run_bass_kernel_spmd(nc, in_maps, list(range(NCORES)), trace=trace)


def kernel(input_ids, embed, w_inner, w_head, phi, b,
           alpha_pre, alpha_post, alpha_res):
    in_maps = _make_in_maps(input_ids, embed, w_inner, w_head, phi, b,
                            alpha_pre, alpha_post, alpha_res)
    res = _run(in_maps).results
    out = np.concatenate([np.asarray(res[i]["out"]) for i in range(NCORES)],
                         axis=1)[:, :VOCAB]
    return np.ascontiguousarray(out.reshape(B, S, VOCAB).astype(np.float32))
